# revision 61
# baseline (speedup 1.0000x reference)
"""Trainium2 Bass kernel for a 12-head causal attention block (GPT-2 style).

Problem: x:[4,2048,768] -> qkv = x@W_attn+b_attn, causal softmax attention
(12 heads, d=64), out @ W_proj + b_proj.

Sharding over 8 NeuronCores: core c handles batch b=c//2 (data parallel) and
head-group hg=c%2 (6 heads = 3 head-pairs, tensor parallel on the qkv
columns / proj rows).  Each core returns a partial projection output; the
host sums the two head-group partials per batch and adds b_proj.

v5 design (~201us, from the 226us v2 baseline; trace-driven changes):
  - final-group normalize: reciprocal broadcast via a rank-1 PE matmul
    (trimask's all-ones row x recips) into a free scores-PSUM tile instead
    of two serial gpsimd partition_broadcasts; head1 is shifted
    UNNORMALIZED in parallel with the chain and multiplied in place at
    partitions 64-127 reading the PSUM broadcast directly (-5us tail).
  - drain proj tiles t8/t9 split into pairs-0/1 accumulation (runs during
    the normalize chain; t8 borrows the freed AV PSUM banks) + pair-2
    finish, so the strict-FIFO PE queue no longer parks on the chain.
  - AV-PSUM evacuation deferred to the NEXT group's first step: emitted
    inline it sat AHEAD of that group's first exp in the engine FIFO
    (+1.2us stall at pair boundaries); deferred it queues behind the exp
    but still before av(0) needs the banks (-3us).
  - critical lead-in loads carry only the m0/m3 qT/kT columns (pair0-v
    deferred; it is not read until ~19us), in chunk halves.
  - deadline-queue tuning: LOOKAHEAD 9 -> 32 and up to TWO background
    units pulled per step (the 1-unit/step spread over-throttled the PE's
    background matmul supply; swept 9/12/16/24/32/48, optimum 32);
    deeper SBUF pools (pt/avsb/rc/bc/ystage) to loosen WAR coupling.
  - exp on deep-diagonal tiles (c0>=256) split into the two live per-head
    ranges (head1's [512:512+c0] was computed but never read).
  - causal masking is a DVE multiply by a precomputed 128x128 triangle
    tile instead of gpsimd affine_select: the affine_selects queued behind
    the previous pair's normalize partition_broadcast on the strict-FIFO
    gpsimd queue, stalling the next pair's first AV ~5-6us at every pair
    boundary (the single biggest win, -9us).
  - inputs are HOST-PACKED so every transfer is a contiguous 2D DMA with
    few issues: xT quarter-major [128, 4*3072], w_qkv split into the
    first-group-critical column blocks {m0-qT, m3-kT, pair0-v} (cols
    0:2304) and the rest; the two critical loads are split in halves on
    the sync+scalar HWDGE rings so the first qk matmul starts as soon as
    the first chunks land.  (The naive [768,2048] xT layout cost a
    768-descriptor strided transfer on the critical path.)
  - normalize: single [65,1024] avsb staging tile for both heads (one
    denominator-reshape DMA, one reciprocal-redistribute DMA, both on
    sync); avsb evacuation runs on ACT for the last group of each pair
    (ACT idles at boundaries, DVE is congested).
  - pair-2 normalize/proj deadlines tightened so each group's proj tiles
    emit during the NEXT group's j-loop, and the drain-phase proj stage
    copies run on ACT; v2 left ~2 groups of proj work after the last exp.
  - qT/kT for pairs 1/2 are emitted 3 steps early (group-boundary exp
    stalls); pair 0's stay just-in-time because its xT quarters are still
    in flight and emitting compute against un-landed DMAs parks PSUM
    buffers and the strict-FIFO PE queue (measured +40us when tried).
  - y partials stored in bf16 (host sums in fp32), batched 2 seq-tiles
    per DMA; ones/bias_v DMAs skipped when the v bias is zero.

  Measured-dead-ends kept out: fp8 (e4m3 per-element quantization error
  does not average out in zero-mean dots -> ~4-6%% output error vs the 2%%
  budget), AV K-split row-tiling (hardware hang), walrus
  --enable-ldw-opt=true (codegen crash), gpsimd SWDGE for bulk input DMAs
  (+40us).
"""

import os
import ml_dtypes
import numpy as np

N_HEAD = 12
N_EMBD = 768
HEAD_DIM = 64
B, S = 4, 2048
N_CORES = 8
HG_HEADS = 6            # heads per core (3 pairs)
HG_DIM = HG_HEADS * HEAD_DIM   # 384
QKV_W = 3 * HG_DIM      # 1152 qkv columns per core
N_PAIRS = 3
ST = S // 128           # 16 seq tiles of 128
NG = S // 512           # 4 seq groups of 512

LAST_RESULTS = None
_PROGRAMS = {}


def _build_program(skip_vbias=False):
    import concourse.bacc as bacc
    import concourse.tile as tile
    from concourse import mybir


    F32 = mybir.dt.float32
    BF16 = mybir.dt.bfloat16
    AF = mybir.ActivationFunctionType

    nc = bacc.Bacc(None, target_bir_lowering=False)
    # host-packed xT, [128, 12288]: col g*3072 + k*512 + s holds
    # xT[k*128+p, g*512+s] -- each 512-seq quarter is one contiguous 2D DMA
    # (the naive [768,2048] layout needed a 768-descriptor strided transfer
    # that sat on the critical path for ~7us).
    xT_d = nc.declare_dram_parameter("xT", [128, 4 * 3072], BF16, isOutput=False)
    # host-packed qkv weights, [128, 6912]: cols 0:2304 hold the
    # first-attention-group-critical blocks {m0-qT, m3-kT, pair0-v} k-major
    # (384 per k-chunk), cols 2304:6912 the complement {m1, m2, m4, m5,
    # v1, v2} k-major (768 per k-chunk) -- so the critical lead-in load and
    # the deferred load are ONE contiguous 2D DMA each.
    wqkv_d = nc.declare_dram_parameter("w_qkv", [128, 54 * 128], BF16, isOutput=False)
    bqk_d = nc.declare_dram_parameter("b_qk", [768], F32, isOutput=False)
    bv_d = nc.declare_dram_parameter("b_v", [HG_DIM], BF16, isOutput=False)
    wproj_d = nc.declare_dram_parameter("w_proj", [HG_DIM, N_EMBD], BF16, isOutput=False)
    ones_d = nc.declare_dram_parameter("ones", [1, 128], BF16, isOutput=False)
    # y partials in bf16: halves the store traffic (the host sums the two
    # head-group partials in fp32; bf16 partial rounding adds ~0.1% error)
    y_d = nc.declare_dram_parameter("y", [S, N_EMBD], BF16, isOutput=True)

    with tile.TileContext(nc) as tc:
        from contextlib import ExitStack

        with ExitStack() as outer:
            consts = outer.enter_context(tc.tile_pool(name="consts", bufs=1))
            # scratch operand for the PE warm-up matmuls below; its memset
            # leads the gpsimd queue so the dummies can start right after
            # the preamble
            warm = consts.tile([128, 512], BF16)
            nc.gpsimd.memset(warm[:], 1.0)
            ones_row = consts.tile([1, 128], BF16)
            bias_v = consts.tile([1, HG_DIM], BF16)
            if not skip_vbias:
                nc.gpsimd.dma_start(out=ones_row[:], in_=ones_d[:])
                nc.gpsimd.dma_start(
                    out=bias_v[:], in_=bv_d[0:HG_DIM].rearrange("(o v) -> o v", o=1)
                )
            bias_qk = consts.tile([128, 6], F32)      # col m: b_qk[128m:128m+128]
            nc.gpsimd.dma_start(
                out=bias_qk[:], in_=bqk_d[0:768].rearrange("(m p) -> p m", p=128)
            )

            # ---- persistent activations/weights in SBUF (all bf16) ----
            big = outer.enter_context(tc.tile_pool(name="big", bufs=1))
            xT = big.tile([128, 6 * S], BF16)       # [emb-part, k-chunk*2048+seq]
            w_all = big.tile([128, 54 * 128], BF16)  # packed layout (see wqkv_d)

            def wcol(k, which):
                # column of 128-wide weight block `which` of k-chunk k in the
                # packed w_all layout: {m0,m3} k-major (cols 0:1536), then
                # pair0-v blocks (1536:2304), then the rest
                if which == "m0":
                    return k * 256
                if which == "m3":
                    return k * 256 + 128
                if which == "v0":
                    return 1536 + k * 128
                ri = {"m1": 0, "m2": 1, "m4": 2, "m5": 3, "v1": 4, "v2": 5}
                return 2304 + k * 768 + ri[which] * 128
            w_proj = big.tile([128, N_PAIRS * N_EMBD], BF16)
            qkT = big.tile([128, 6 * S], BF16)      # m=0..2 qT pairs, m=3..5 kT pairs
            # per k-tile: 6 heads x (64 v-cols + a ones col for the softmax
            # denominator) -> P@V and row-sums come from one M=65 matmul
            v_all = big.tile([128, ST * 390], BF16)  # [seq, t*390 + 65h + d]
            attnT = big.tile([128, N_PAIRS * S], BF16)

            nc.gpsimd.memset(v_all[:], 1.0)
            # causal 128x128 triangle mask (1 where q-col >= k-row), built
            # once: the per-diag-tile masking is a DVE multiply by this tile
            # instead of a gpsimd affine_select -- affine_selects queued
            # BEHIND the previous pair's normalize partition_broadcast on the
            # strict-FIFO gpsimd at every pair boundary, stalling av(j0) ~5us.
            trimask = consts.tile([128, 128], BF16)
            nc.gpsimd.memset(trimask[:], 1.0)
            nc.gpsimd.affine_select(
                out=trimask[:], in_=trimask[:],
                compare_op=mybir.AluOpType.is_ge,
                fill=0.0, base=0, pattern=[[1, 128]], channel_multiplier=-1,
            )
            # CRITICAL lead-in inputs as SINGLE multi-dim strided DMAs (each
            # dma_start costs ~0.6us of ISSUE time on its trigger engine, so
            # issue count is what matters): the w columns the first attention
            # group needs ({0:128 m0-qT, 384:512 m3-kT, 768:896 pair0-v} per
            # k-chunk) in one DMA on sync, and the xT g0 quarter (cols 0:512
            # of every k-chunk) in one DMA on scalar, in parallel.
            # qT/kT weights first ({m0,m3}, in chunk halves so the first
            # matmuls start as soon as chunks 0-2 land), then the pair0-v
            # blocks (not read until av(j=0) at ~19us)
            nc.sync.dma_start(out=w_all[:, 0:768], in_=wqkv_d[:, 0:768])
            nc.sync.dma_start(out=w_all[:, 768:1536], in_=wqkv_d[:, 768:1536])
            nc.sync.dma_start(out=w_all[:, 1536:2304], in_=wqkv_d[:, 1536:2304])
            xT_view_s = xT[:].rearrange("p (k s) -> p k s", k=6)
            nc.scalar.dma_start(out=xT_view_s[:, 0:3, 0:512],
                                in_=xT_d[:, 0:1536])
            nc.scalar.dma_start(out=xT_view_s[:, 3:6, 0:512],
                                in_=xT_d[:, 1536:3072])


            # deferred inputs (one contiguous DMA each), deadline-queued on
            # sync behind the critical lead-in transfers.
            def emit_w_rest():
                nc.sync.dma_start(out=w_all[:, 2304:6912], in_=wqkv_d[:, 2304:6912])

            def emit_xT_quarter(g):
                nc.sync.dma_start(out=xT_view_s[:, :, g * 512:(g + 1) * 512],
                                  in_=xT_d[:, g * 3072:(g + 1) * 3072])

            def emit_wproj():
                nc.sync.dma_start(
                    out=w_proj[:].rearrange("p (c e) -> p c e", c=3),
                    in_=wproj_d[:].rearrange("(c p) e -> p c e", p=128),
                )

            # ---- pools ----
            stps = outer.enter_context(tc.tile_pool(name="stps", bufs=2, space="PSUM"))
            avps = outer.enter_context(tc.tile_pool(name="avps", bufs=2, space="PSUM"))
            auxps = outer.enter_context(tc.tile_pool(name="auxps", bufs=2, space="PSUM"))
            ptp = outer.enter_context(tc.tile_pool(name="ptp", bufs=6))
            avsb = outer.enter_context(tc.tile_pool(name="avsb", bufs=4))
            rcp = outer.enter_context(tc.tile_pool(name="rcp", bufs=6))
            bcp = outer.enter_context(tc.tile_pool(name="bcp", bufs=6))
            shtmp = outer.enter_context(tc.tile_pool(name="shtmp", bufs=3))
            ystage = outer.enter_context(tc.tile_pool(name="ystage", bufs=3))

            # PE clock warm-up: the HAM gate keeps the PE at 1.2 GHz until
            # ~3.4us of sustained activity.  The PE is otherwise idle from
            # the preamble (~7us) until the critical DMAs land (~14us), so
            # burn that window on dummy matmuls over scratch data -- the
            # first REAL qk chains then run at 2.4 GHz (were ~5us cold).
            wps = auxps.tile([128, 512], F32, tag="aux")
            for _ in range(9):
                nc.tensor.matmul(wps[:], warm[:, 0:128], warm[:],
                                 start=True, stop=True)

            v_view = v_all[:].rearrange("p (t h c) -> p t h c", t=ST, h=HG_HEADS)

            # ---- work-unit emitters (each emits a small PE-dense chunk) ----
            def emit_qk_group(m, g):
                # qkT[:, m*S + g*512 : +512] = (W[:, m-block].T @ xT)[:, g-block] + bias
                ps = auxps.tile([128, 512], F32, tag="aux")
                for k in range(6):
                    wc = wcol(k, f"m{m}")
                    nc.tensor.matmul(
                        ps[:],
                        w_all[:, wc:wc + 128],
                        xT[:, k * S + g * 512:k * S + (g + 1) * 512],
                        start=(k == 0), stop=(k == 5),
                    )
                nc.vector.tensor_scalar_add(
                    qkT[:, m * S + g * 512:m * S + (g + 1) * 512],
                    ps[:], bias_qk[:, m:m + 1],
                )

            def emit_v_tile(pair, t):
                # v rows t*128.. for this pair's two heads (N=128); split by
                # pair so each attention slot computes only its own v work
                ps = auxps.tile([128, 128], F32, tag="aux")
                for k in range(6):
                    wc = wcol(k, f"v{pair}")
                    nc.tensor.matmul(
                        ps[:],
                        xT[:, k * S + t * 128:k * S + (t + 1) * 128],
                        w_all[:, wc:wc + 128],
                        start=(k == 0), stop=(skip_vbias and k == 5),
                    )
                if not skip_vbias:
                    nc.tensor.matmul(   # += ones^T[1,128].T @ bias_v[1,128]
                        ps[:], ones_row[:],
                        bias_v[:, pair * 128:(pair + 1) * 128],
                        start=False, stop=True,
                    )
                nc.vector.tensor_copy(
                    v_view[:, t, 2 * pair:2 * pair + 2, 0:64],
                    ps[:].rearrange("p (h d) -> p h d", h=2),
                )

            ys_pending = {}
            drain_ps = {}

            def emit_proj_partial(t, use_avps):
                # drain phase: pairs 0/1 of a proj tile accumulate while the
                # final normalize chain resolves (pair 2 would block the
                # strict-FIFO PE queue).  Tile t8 borrows the now-free AV
                # PSUM banks so two tiles can be in flight alongside the
                # aux pool.
                pool = avps if use_avps else auxps
                tag = "av" if use_avps else "aux"
                psA = pool.tile([128, 512], F32, tag=tag)
                psB = pool.tile([128, 256], F32, tag=tag)
                for p in range(2):
                    lhsT = attnT[:, p * S + t * 128:p * S + (t + 1) * 128]
                    nc.tensor.matmul(psA[:], lhsT, w_proj[:, p * N_EMBD:p * N_EMBD + 512],
                                     start=(p == 0), stop=False)
                    nc.tensor.matmul(psB[:], lhsT,
                                     w_proj[:, p * N_EMBD + 512:(p + 1) * N_EMBD],
                                     start=(p == 0), stop=False)
                drain_ps[t] = (psA, psB)

            def emit_proj_finish(t):
                psA, psB = drain_ps.pop(t)
                lhsT = attnT[:, 2 * S + t * 128:2 * S + (t + 1) * 128]
                nc.tensor.matmul(psA[:], lhsT, w_proj[:, 2 * N_EMBD:2 * N_EMBD + 512],
                                 start=False, stop=True)
                nc.tensor.matmul(psB[:], lhsT,
                                 w_proj[:, 2 * N_EMBD + 512:3 * N_EMBD],
                                 start=False, stop=True)
                ys = ystage.tile([128, 2 * N_EMBD], BF16, tag="ys")
                AFc = mybir.ActivationFunctionType.Copy
                nc.scalar.activation(ys[:, 0:512], psA[:], AFc)
                nc.scalar.activation(ys[:, 512:768], psB[:], AFc)
                nc.sync.dma_start(out=y_d[t * 128:(t + 1) * 128, :],
                                  in_=ys[:, 0:768])

            def emit_proj_tile(t, drain=False):
                # stage into the left/right half of a 2-tile ystage buffer;
                # the odd tile of each pair issues one batched y DMA.  In the
                # post-exp drain the PSUM->stage copies run on the (now idle)
                # ACT engine so they never queue behind DVE normalize work.
                psA = auxps.tile([128, 512], F32, tag="aux")
                psB = auxps.tile([128, 256], F32, tag="aux")
                for p in range(N_PAIRS):
                    lhsT = attnT[:, p * S + t * 128:p * S + (t + 1) * 128]
                    nc.tensor.matmul(psA[:], lhsT, w_proj[:, p * N_EMBD:p * N_EMBD + 512],
                                     start=(p == 0), stop=(p == N_PAIRS - 1))
                    nc.tensor.matmul(psB[:], lhsT,
                                     w_proj[:, p * N_EMBD + 512:(p + 1) * N_EMBD],
                                     start=(p == 0), stop=(p == N_PAIRS - 1))
                if drain:
                    # drain phase: per-tile stores (a 2-tile batch would hold
                    # the last store until both tiles finish) and ACT copies
                    # (the DVE is busy with the final normalize)
                    ys = ystage.tile([128, 2 * N_EMBD], BF16, tag="ys")
                    AFc = mybir.ActivationFunctionType.Copy
                    nc.scalar.activation(ys[:, 0:512], psA[:], AFc)
                    nc.scalar.activation(ys[:, 512:768], psB[:], AFc)
                    nc.sync.dma_start(out=y_d[t * 128:(t + 1) * 128, :],
                                      in_=ys[:, 0:768])
                    return
                if t % 2 == 0:
                    ys = ystage.tile([128, 2 * N_EMBD], BF16, tag="ys")
                    ys_pending[t] = ys
                else:
                    ys = ys_pending.pop(t - 1)
                half = (t % 2) * N_EMBD
                nc.vector.tensor_copy(ys[:, half:half + 512], psA[:])
                nc.vector.tensor_copy(ys[:, half + 512:half + 768], psB[:])
                if t % 2 == 1:
                    b = t // 2
                    nc.sync.dma_start(
                        out=y_d[b * 256:(b + 1) * 256, :]
                            .rearrange("(i p) e -> p i e", p=128),
                        in_=ys[:].rearrange("p (i e) -> p i e", i=2),
                    )

            # ---- deadline-driven background work queue ----
            # Attention groups execute in a fixed order; (pair, g, j) maps to
            # a global step.  Each qkv/proj work unit carries the step by
            # which it MUST be emitted (Tile deps are emission-order-based:
            # a read emitted before its producer gets no dependency).  Units
            # are pulled with LOOKAHEAD steps of slack so the PE always has
            # background matmuls to chew on while ACT runs exp.
            # pair-2 groups run [1,0,3,2]: each group's normalize + proj
            # tiles emit early in the FOLLOWING group (tight deadlines), so
            # after the last exp only group g2's normalize + proj t8-11
            # remain.
            group_order = {0: [0, 1, 2, 3], 1: [0, 1, 2, 3], 2: [1, 0, 3, 2]}
            step_base = {}
            _acc = 0
            for _p in range(N_PAIRS):
                for _g in group_order[_p]:
                    step_base[(_p, _g)] = _acc
                    _acc += 4 * _g + 4
            TOTAL_STEPS = _acc
            LOOKAHEAD = 32

            work_q = []   # sorted list of (deadline_step, seq, fn)
            _seq = [0]

            def push(deadline, fn):
                import bisect
                _seq[0] += 1
                bisect.insort(work_q, (deadline, _seq[0], fn))

            def pull_work(cur_step):
                # overdue units MUST emit now (correctness: emission order
                # defines Tile dependencies); otherwise spread at one unit
                # per step so the background work stays evenly interleaved.
                while work_q and work_q[0][0] <= cur_step:
                    work_q.pop(0)[2]()
                for _ in range(2):
                    if work_q and work_q[0][0] <= cur_step + LOOKAHEAD:
                        work_q.pop(0)[2]()

            # ---- attention group with interleaved background units ----
            sts_all = {}
            pts_all = {}

            def scores_pg(pair, g, j):
                q0 = pair * S
                k0 = (3 + pair) * S
                diag_r = j - 4 * g
                c0 = 128 * diag_r if diag_r >= 0 else 0
                st = stps.tile([128, 1024], F32, tag="st")
                nc.tensor.matmul(
                    st[:, c0:512],
                    qkT[0:64, k0 + j * 128:k0 + (j + 1) * 128],
                    qkT[0:64, q0 + g * 512 + c0:q0 + (g + 1) * 512],
                    start=True, stop=True, tile_position=(0, 0),
                )
                nc.tensor.matmul(
                    st[:, 512 + c0:1024],
                    qkT[64:128, k0 + j * 128:k0 + (j + 1) * 128],
                    qkT[64:128, q0 + g * 512 + c0:q0 + (g + 1) * 512],
                    start=True, stop=True, tile_position=(64, 0),
                )
                sts_all[(pair, g, j)] = (st, c0)

            def expmask_pg(pair, g, j):
                st, c0 = sts_all.pop((pair, g, j))
                pt = ptp.tile([128, 1024], BF16, tag="pt")
                if c0 >= 256:
                    # deep-diagonal tile: head1's [512:512+c0] range is never
                    # read by its AV matmul, so exp the two live ranges
                    # separately (saves c0*128 ACT elements, > the ~170ns
                    # extra instruction cost once c0 >= 256)
                    nc.scalar.activation(pt[:, c0:512], st[:, c0:512],
                                         AF.Exp, bias=0.0, scale=0.125)
                    nc.scalar.activation(pt[:, 512 + c0:1024], st[:, 512 + c0:1024],
                                         AF.Exp, bias=0.0, scale=0.125)
                else:
                    nc.scalar.activation(pt[:, c0:1024], st[:, c0:1024],
                                         AF.Exp, bias=0.0, scale=0.125)
                diag_r = j - 4 * g
                if diag_r >= 0:
                    with nc.allow_low_precision(reason="bf16 causal mask"):
                        for h in range(2):
                            nc.vector.tensor_mul(
                                pt[:, h * 512 + c0:h * 512 + c0 + 128],
                                pt[:, h * 512 + c0:h * 512 + c0 + 128],
                                trimask[:],
                            )
                pts_all[(pair, g, j)] = (pt, c0)

            def emit_attn_group(pair, g, nxt_ctx=None):
                njt = 4 * g + 4
                av0 = avps.tile([65, 512], F32, tag="av")
                av1 = avps.tile([65, 512], F32, tag="av")

                def av(j):
                    pt, c0 = pts_all.pop((pair, g, j))
                    first, last = (j == 0), (j == njt - 1)
                    for h, avt in ((0, av0), (1, av1)):
                        nc.tensor.matmul(
                            avt[0:65, c0:512],
                            v_all[:, j * 390 + (2 * pair + h) * 65:
                                  j * 390 + (2 * pair + h) * 65 + 65],
                            pt[:, h * 512 + c0:(h + 1) * 512],
                            start=first, stop=last,
                        )

                if (pair, g, 0) not in pts_all:
                    scores_pg(pair, g, 0)
                    expmask_pg(pair, g, 0)
                base = step_base[(pair, g)]
                for j in range(njt):
                    if j + 1 < njt:
                        scores_pg(pair, g, j + 1)
                        expmask_pg(pair, g, j + 1)
                    pull_work(base + j)
                    av(j)

                # evacuate the AV accumulators to SBUF (one copy per head into
                # a shared [65,1024] staging tile -- frees the PSUM banks for
                # the next group's AV almost immediately); the
                # recip/redistribute/multiply chain is DEFERRED into the next
                # group's instruction stream so it never stalls the PE at the
                # group boundary.
                avs = avsb.tile([65, 1024], F32, tag="avsb")

                def evacuate():
                    if g == group_order[pair][-1]:
                        # pair boundary: evacuate on ACT (it idles there
                        # while the DVE works the normalize chain)
                        AFc = mybir.ActivationFunctionType.Copy
                        nc.scalar.activation(avs[:, 0:512], av0[:], AFc)
                        nc.scalar.activation(avs[:, 512:1024], av1[:], AFc)
                    else:
                        nc.vector.tensor_copy(avs[:, 0:512], av0[:])
                        nc.vector.tensor_copy(avs[:, 512:1024], av1[:])

                final = (pair == 2 and g == group_order[2][-1])
                if final:
                    evacuate()
                else:
                    # deferred to the NEXT group's step 0: the copies then
                    # queue BEHIND its first exp in the engine FIFO (emitted
                    # inline they delayed that exp ~1.2us at pair
                    # boundaries) but still ahead of its av(0), which needs
                    # these PSUM banks back.
                    push(base + njt, evacuate)

                # with the consolidated input DMAs the sync queue is near
                # idle mid-kernel, so all normalize DMAs ride it (HWDGE; the
                # gpsimd SWDGE path costs ~1us + library reloads and stalled
                # the chain behind affine_selects in practice).
                dma_eng = nc.sync

                def normalize():
                    cols = slice(pair * S + g * 512, pair * S + (g + 1) * 512)
                    # DVE reciprocal runs ~9 cyc/elem PER LANE: on [1,1024]
                    # it would cost ~6us.  Reshape both heads' denominators
                    # to [128,8] via ONE SBUF DMA (flat row-major pairing:
                    # partition p <- cols 8p..8p+7, so p<64 is head0) so the
                    # recip uses 128 lanes (~0.2us), then shape back to
                    # [1,1024] for the gpsimd partition broadcasts.
                    dn8 = rcp.tile([128, 8], F32, tag="dn8")
                    dma_eng.dma_start(out=dn8[:], in_=avs[64:65, :])
                    if final:
                        # head1's UNNORMALIZED rows shifted to partitions
                        # 64-127 in parallel with the reciprocal chain; the
                        # in-place multiply below reads the PE broadcast at
                        # partitions 64-127 directly, cutting the
                        # mult->shift->sem tail (~2.6us measured) off the
                        # final critical path.
                        sh = shtmp.tile([128, 512], F32, tag="sh")
                        dma_eng.dma_start(out=sh[64:128, :],
                                          in_=avs[0:64, 512:1024])
                    with nc.allow_low_precision(reason="softmax normalize bf16"):
                        if final:
                            # drain phase: the PE is idle and the scores PSUM
                            # pool is free, so broadcast the reciprocals with
                            # a rank-1 matmul (trimask row 0 is all-ones)
                            # instead of two serial ~1us partition_broadcasts
                            # -- and head1's multiply reads the PSUM
                            # broadcast directly (every row is identical), so
                            # no gpsimd at all on the final critical chain.
                            rc8b = rcp.tile([128, 8], BF16, tag="rc8b")
                            nc.vector.reciprocal(rc8b[:], dn8[:])
                            rc2b = rcp.tile([1, 1024], BF16, tag="rc2b")
                            dma_eng.dma_start(out=rc2b[:], in_=rc8b[:])
                            bc_ps = stps.tile([128, 1024], F32, tag="st")
                            for h in range(2):
                                nc.tensor.matmul(
                                    bc_ps[:, h * 512:(h + 1) * 512],
                                    trimask[0:1, 0:128],
                                    rc2b[:, h * 512:(h + 1) * 512],
                                    start=True, stop=True,
                                )
                            nc.vector.tensor_mul(attnT[0:64, cols],
                                                 avs[0:64, 0:512],
                                                 bc_ps[0:64, 0:512])
                            nc.vector.tensor_mul(attnT[64:128, cols],
                                                 sh[64:128, :],
                                                 bc_ps[64:128, 512:1024])
                            return
                        rc8 = rcp.tile([128, 8], F32, tag="rc8")
                        nc.vector.reciprocal(rc8[:], dn8[:])
                        rc2 = rcp.tile([1, 1024], F32, tag="rc2")
                        dma_eng.dma_start(out=rc2[:], in_=rc8[:])
                        for h in range(2):
                            bc = bcp.tile([64, 512], F32)
                            nc.gpsimd.partition_broadcast(
                                bc[:], rc2[:, h * 512:(h + 1) * 512], channels=64)
                            if h == 0:
                                nc.vector.tensor_mul(attnT[0:64, cols],
                                                     avs[0:64, 0:512], bc[:])
                            else:
                                # DVE lanes are partition-locked: odd head's
                                # rows 64-127 via an SBUF bounce + DMA shift
                                tmp = shtmp.tile([64, 512], BF16)
                                nc.vector.tensor_mul(tmp[:], avs[0:64, 512:1024],
                                                     bc[:])
                                nc.sync.dma_start(out=attnT[64:128, cols],
                                                  in_=tmp[:])

                nxt = base + njt
                if pair == 2:
                    # tight deadlines: normalize pops at the next group's
                    # step 0 (eligible from nxt+1-LOOKAHEAD, head of queue by
                    # (deadline, seq)), proj tiles follow one per step.  For
                    # the final group nxt == TOTAL_STEPS and these drain
                    # immediately after the j-loop, in push order.
                    push(nxt + 1, normalize)
                    if final:
                        t0 = 4 * g
                        push(nxt + 2, lambda: emit_proj_partial(t0, True))
                        push(nxt + 3, lambda: emit_proj_partial(t0 + 1, False))
                        push(nxt + 4, lambda: emit_proj_finish(t0))
                        push(nxt + 5, lambda: emit_proj_finish(t0 + 1))
                        push(nxt + 6, lambda: emit_proj_tile(t0 + 2, drain=True))
                        push(nxt + 7, lambda: emit_proj_tile(t0 + 3, drain=True))
                    else:
                        for i, t in enumerate(range(4 * g, 4 * g + 4)):
                            push(nxt + 2 + i, lambda t=t: emit_proj_tile(t))
                else:
                    # pairs 0/1: keep the relaxed deadline so the broadcast
                    # queues behind the next group's first affine_selects.
                    push(nxt + LOOKAHEAD, normalize)

            # ================= schedule =================
            # upfront: just enough qkv for attn(0, g0); v t0-3 go through
            # the deadline queue (first read at av(j=t) of group (0,0))
            emit_qk_group(3, 0)          # kT pair 0, seq 0-511
            emit_qk_group(0, 0)          # qT pair 0, seq 0-511

            # deadlines: qT(p, g) is read only by group (p, g); kT(p, g') is
            # read by EVERY group (p, g >= g'), so its deadline is the
            # earliest-executing such group - for pair 2 (non-monotone group
            # order) that is the first group of the pair for ALL kT chunks.
            # qT/kT for pairs 1/2 run THREE steps early: emitted
            # just-in-time (base-1) the next group's first scores wait ~2us
            # for the qk chain + bias add, stalling the exp stream at every
            # group boundary.  Pair 0's stay just-in-time: its xT quarters
            # are still IN FLIGHT, and emitting compute against an un-landed
            # DMA parks a PSUM buffer + the strict-FIFO PE queue on it
            # (measured +40us!).
            for p in range(N_PAIRS):
                for g in range(NG):
                    if (p, g) == (0, 0):
                        continue
                    slack = 3 if p > 0 else 1
                    kt_dl = min(step_base[(p, gg)] for gg in range(g, NG)) - slack
                    push(kt_dl, lambda m=3 + p, g=g: emit_qk_group(m, g))
                    push(step_base[(p, g)] - slack,
                         lambda m=p, g=g: emit_qk_group(m, g))
            # v(pair, t) is first read at av(j=t) of the earliest-executing
            # group g of that pair with 4g+3 >= t
            for p in range(N_PAIRS):
                for t in range(16):
                    dl = min(step_base[(p, g)]
                             for g in group_order[p] if 4 * g + 3 >= t) + t
                    push(dl, lambda p=p, t=t: emit_v_tile(p, t))
            # deferred w complement: needed first by pair-1 qT/kT/v work
            # (earliest deadline around step_base[(1,0)]-3)
            push(step_base[(0, 2)], emit_w_rest)
            # xT quarter g is first read by qk(0, g) units (deadline base-1)
            for g in range(1, NG):
                push(step_base[(0, g)] - 2, lambda g=g: emit_xT_quarter(g))
            # w_proj is first read by proj units in pair 2
            push(step_base[(1, 0)], emit_wproj)

            seq = [(p, g) for p in range(N_PAIRS) for g in group_order[p]]
            for i, (pair, g) in enumerate(seq):
                nxt_ctx = seq[i + 1] if i + 1 < len(seq) else None
                emit_attn_group(pair, g, nxt_ctx)

            # drain in deadline order: the final group's normalize precedes
            # its proj tiles (same-ordered deadlines)
            while work_q:
                work_q.pop(0)[2]()

    nc.compile()
    return nc


def _numpy_fallback(x, mask, W_attn, b_attn, W_proj, b_proj):
    qkv = x @ W_attn + b_attn
    q, k, v = np.split(qkv, 3, axis=-1)

    def heads(t):
        return t.reshape(B, S, N_HEAD, HEAD_DIM).transpose(0, 2, 1, 3)

    q, k, v = heads(q), heads(k), heads(v)
    attn = np.einsum("bhqd,bhkd->bhqk", q, k) / np.sqrt(np.float32(HEAD_DIM))
    attn = attn + mask * (-1e9)
    attn = attn - attn.max(axis=-1, keepdims=True)
    attn = np.exp(attn)
    attn = attn / attn.sum(axis=-1, keepdims=True)
    out = np.einsum("bhqk,bhkd->bhqd", attn, v)
    out = out.transpose(0, 2, 1, 3).reshape(B, S, N_EMBD)
    return (out @ W_proj + b_proj).astype(np.float32)


def _pack_w(Wc):
    """[768, 1152] per-core qkv weight -> [128, 6912] packed layout: cols
    0:2304 = k-major {m0, m3, v0} blocks (the first attention group's
    critical columns), cols 2304:6912 = k-major {m1, m2, m4, m5, v1, v2}."""
    critA = np.concatenate([Wc[:, 0:128], Wc[:, 384:512]], axis=1)  # [768, 256]
    critAP = critA.reshape(6, 128, 256).transpose(1, 0, 2).reshape(128, 1536)
    vblkP = Wc[:, 768:896].reshape(6, 128, 128).transpose(1, 0, 2).reshape(128, 768)
    rest = np.concatenate(
        [Wc[:, 128:384], Wc[:, 512:768], Wc[:, 896:1152]], axis=1)   # [768, 768]
    restP = rest.reshape(6, 128, 768).transpose(1, 0, 2).reshape(128, 4608)
    return np.concatenate([critAP, vblkP, restP], axis=1)


def make_in_maps(x, W_attn, b_attn, W_proj):
    bf16 = ml_dtypes.bfloat16
    in_maps = []
    for c in range(N_CORES):
        b, hg = divmod(c, 2)
        o = HG_DIM * hg
        Wc = np.concatenate(
            [W_attn[:, o:o + HG_DIM],
             W_attn[:, 768 + o:768 + o + HG_DIM],
             W_attn[:, 1536 + o:1536 + o + HG_DIM]], axis=1)
        xTc = x[b].T.astype(bf16)   # [768, 2048]
        xT_packed = (xTc.reshape(6, 128, 4, 512).transpose(1, 2, 0, 3)
                     .reshape(128, 4 * 3072))
        in_maps.append({
            "xT": np.ascontiguousarray(xT_packed),
            "w_qkv": np.ascontiguousarray(_pack_w(Wc).astype(bf16)),
            "b_qk": np.ascontiguousarray(np.concatenate(
                [b_attn[o:o + HG_DIM], b_attn[768 + o:768 + o + HG_DIM]])),
            "b_v": np.ascontiguousarray(b_attn[1536 + o:1536 + o + HG_DIM]).astype(bf16),
            "w_proj": np.ascontiguousarray(W_proj[o:o + HG_DIM, :].astype(bf16)),
            "ones": np.ones((1, 128), dtype=bf16),
        })
    return in_maps


def kernel(x, mask, W_attn, b_attn, W_proj, b_proj):
    global LAST_RESULTS
    x = np.asarray(x, dtype=np.float32)
    mask = np.asarray(mask, dtype=np.float32)
    W_attn = np.asarray(W_attn, dtype=np.float32)
    b_attn = np.asarray(b_attn, dtype=np.float32)
    W_proj = np.asarray(W_proj, dtype=np.float32)
    b_proj = np.asarray(b_proj, dtype=np.float32)

    # the kernel exploits causal structure; verify the mask actually is causal
    causal = 1.0 - np.tril(np.ones((S, S), dtype=np.float32))
    if mask.shape != (1, 1, S, S) or not np.array_equal(mask[0, 0], causal):
        return _numpy_fallback(x, mask, W_attn, b_attn, W_proj, b_proj)

    from concourse.bass_utils import run_bass_kernel_spmd

    skip_vbias = not b_attn[1536:2304].any()   # v-bias exactly zero
    if skip_vbias not in _PROGRAMS:
        _PROGRAMS[skip_vbias] = _build_program(skip_vbias=skip_vbias)

    in_maps = make_in_maps(x, W_attn, b_attn, W_proj)

    trace = bool(int(os.environ.get("ATTN_KERNEL_TRACE", "0")))
    res = run_bass_kernel_spmd(_PROGRAMS[skip_vbias], in_maps,
                               list(range(N_CORES)), trace=trace)
    LAST_RESULTS = res

    y = np.zeros((B, S, N_EMBD), dtype=np.float32)
    for c in range(N_CORES):
        y[c // 2] += res.results[c]["y"].astype(np.float32)
    y += b_proj
    return y


# revision 62
# speedup vs baseline: 1.0154x; 1.0154x over previous
"""Trainium2 Bass kernel for a 12-head causal attention block (GPT-2 style).

Problem: x:[4,2048,768] -> qkv = x@W_attn+b_attn, causal softmax attention
(12 heads, d=64), out @ W_proj + b_proj.

Sharding over 8 NeuronCores: core c handles batch b=c//2 (data parallel) and
head-group hg=c%2 (6 heads = 3 head-pairs, tensor parallel on the qkv
columns / proj rows).  Each core returns a partial projection output; the
host sums the two head-group partials per batch and adds b_proj.

v5 design (~201us, from the 226us v2 baseline; trace-driven changes):
  - final-group normalize: reciprocal broadcast via a rank-1 PE matmul
    (trimask's all-ones row x recips) into a free scores-PSUM tile instead
    of two serial gpsimd partition_broadcasts; head1 is shifted
    UNNORMALIZED in parallel with the chain and multiplied in place at
    partitions 64-127 reading the PSUM broadcast directly (-5us tail).
  - drain proj tiles t8/t9 split into pairs-0/1 accumulation (runs during
    the normalize chain; t8 borrows the freed AV PSUM banks) + pair-2
    finish, so the strict-FIFO PE queue no longer parks on the chain.
  - AV-PSUM evacuation deferred to the NEXT group's first step: emitted
    inline it sat AHEAD of that group's first exp in the engine FIFO
    (+1.2us stall at pair boundaries); deferred it queues behind the exp
    but still before av(0) needs the banks (-3us).
  - critical lead-in loads carry only the m0/m3 qT/kT columns (pair0-v
    deferred; it is not read until ~19us), in chunk halves.
  - deadline-queue tuning: LOOKAHEAD 9 -> 32 and up to TWO background
    units pulled per step (the 1-unit/step spread over-throttled the PE's
    background matmul supply; swept 9/12/16/24/32/48, optimum 32);
    deeper SBUF pools (pt/avsb/rc/bc/ystage) to loosen WAR coupling.
  - exp on deep-diagonal tiles (c0>=256) split into the two live per-head
    ranges (head1's [512:512+c0] was computed but never read).
  - causal masking is a DVE multiply by a precomputed 128x128 triangle
    tile instead of gpsimd affine_select: the affine_selects queued behind
    the previous pair's normalize partition_broadcast on the strict-FIFO
    gpsimd queue, stalling the next pair's first AV ~5-6us at every pair
    boundary (the single biggest win, -9us).
  - inputs are HOST-PACKED so every transfer is a contiguous 2D DMA with
    few issues: xT quarter-major [128, 4*3072], w_qkv split into the
    first-group-critical column blocks {m0-qT, m3-kT, pair0-v} (cols
    0:2304) and the rest; the two critical loads are split in halves on
    the sync+scalar HWDGE rings so the first qk matmul starts as soon as
    the first chunks land.  (The naive [768,2048] xT layout cost a
    768-descriptor strided transfer on the critical path.)
  - normalize: single [65,1024] avsb staging tile for both heads (one
    denominator-reshape DMA, one reciprocal-redistribute DMA, both on
    sync); avsb evacuation runs on ACT for the last group of each pair
    (ACT idles at boundaries, DVE is congested).
  - pair-2 normalize/proj deadlines tightened so each group's proj tiles
    emit during the NEXT group's j-loop, and the drain-phase proj stage
    copies run on ACT; v2 left ~2 groups of proj work after the last exp.
  - qT/kT for pairs 1/2 are emitted 3 steps early (group-boundary exp
    stalls); pair 0's stay just-in-time because its xT quarters are still
    in flight and emitting compute against un-landed DMAs parks PSUM
    buffers and the strict-FIFO PE queue (measured +40us when tried).
  - y partials stored in bf16 (host sums in fp32), batched 2 seq-tiles
    per DMA; ones/bias_v DMAs skipped when the v bias is zero.

  Measured-dead-ends kept out: fp8 (e4m3 per-element quantization error
  does not average out in zero-mean dots -> ~4-6%% output error vs the 2%%
  budget), AV K-split row-tiling (hardware hang), walrus
  --enable-ldw-opt=true (codegen crash), gpsimd SWDGE for bulk input DMAs
  (+40us).
"""

import os
import ml_dtypes
import numpy as np

N_HEAD = 12
N_EMBD = 768
HEAD_DIM = 64
B, S = 4, 2048
N_CORES = 8
HG_HEADS = 6            # heads per core (3 pairs)
HG_DIM = HG_HEADS * HEAD_DIM   # 384
QKV_W = 3 * HG_DIM      # 1152 qkv columns per core
N_PAIRS = 3
ST = S // 128           # 16 seq tiles of 128
NG = S // 512           # 4 seq groups of 512

LAST_RESULTS = None
_PROGRAMS = {}


def _build_program(skip_vbias=False):
    import concourse.bacc as bacc
    import concourse.tile as tile
    from concourse import mybir


    F32 = mybir.dt.float32
    BF16 = mybir.dt.bfloat16
    AF = mybir.ActivationFunctionType

    nc = bacc.Bacc(None, target_bir_lowering=False)
    # host-packed xT, [128, 12288]: col g*3072 + k*512 + s holds
    # xT[k*128+p, g*512+s] -- each 512-seq quarter is one contiguous 2D DMA
    # (the naive [768,2048] layout needed a 768-descriptor strided transfer
    # that sat on the critical path for ~7us).
    xT_d = nc.declare_dram_parameter("xT", [128, 4 * 3072], BF16, isOutput=False)
    # host-packed qkv weights, [128, 6912]: cols 0:2304 hold the
    # first-attention-group-critical blocks {m0-qT, m3-kT, pair0-v} k-major
    # (384 per k-chunk), cols 2304:6912 the complement {m1, m2, m4, m5,
    # v1, v2} k-major (768 per k-chunk) -- so the critical lead-in load and
    # the deferred load are ONE contiguous 2D DMA each.
    wqkv_d = nc.declare_dram_parameter("w_qkv", [128, 54 * 128], BF16, isOutput=False)
    bqk_d = nc.declare_dram_parameter("b_qk", [768], F32, isOutput=False)
    bv_d = nc.declare_dram_parameter("b_v", [HG_DIM], BF16, isOutput=False)
    wproj_d = nc.declare_dram_parameter("w_proj", [HG_DIM, N_EMBD], BF16, isOutput=False)
    ones_d = nc.declare_dram_parameter("ones", [1, 128], BF16, isOutput=False)
    # y partials in bf16: halves the store traffic (the host sums the two
    # head-group partials in fp32; bf16 partial rounding adds ~0.1% error)
    y_d = nc.declare_dram_parameter("y", [S, N_EMBD], BF16, isOutput=True)

    with tile.TileContext(nc) as tc:
        from contextlib import ExitStack

        with ExitStack() as outer:
            consts = outer.enter_context(tc.tile_pool(name="consts", bufs=1))
            ones_row = consts.tile([1, 128], BF16)
            bias_v = consts.tile([1, HG_DIM], BF16)
            if not skip_vbias:
                nc.gpsimd.dma_start(out=ones_row[:], in_=ones_d[:])
                nc.gpsimd.dma_start(
                    out=bias_v[:], in_=bv_d[0:HG_DIM].rearrange("(o v) -> o v", o=1)
                )
            bias_qk = consts.tile([128, 6], F32)      # col m: b_qk[128m:128m+128]
            nc.gpsimd.dma_start(
                out=bias_qk[:], in_=bqk_d[0:768].rearrange("(m p) -> p m", p=128)
            )

            # ---- persistent activations/weights in SBUF (all bf16) ----
            big = outer.enter_context(tc.tile_pool(name="big", bufs=1))
            xT = big.tile([128, 6 * S], BF16)       # [emb-part, k-chunk*2048+seq]
            w_all = big.tile([128, 54 * 128], BF16)  # packed layout (see wqkv_d)

            def wcol(k, which):
                # column of 128-wide weight block `which` of k-chunk k in the
                # packed w_all layout: {m0,m3} k-major (cols 0:1536), then
                # pair0-v blocks (1536:2304), then the rest
                if which == "m0":
                    return k * 256
                if which == "m3":
                    return k * 256 + 128
                if which == "v0":
                    return 1536 + k * 128
                ri = {"m1": 0, "m2": 1, "m4": 2, "m5": 3, "v1": 4, "v2": 5}
                return 2304 + k * 768 + ri[which] * 128
            w_proj = big.tile([128, N_PAIRS * N_EMBD], BF16)
            qkT = big.tile([128, 6 * S], BF16)      # m=0..2 qT pairs, m=3..5 kT pairs
            # per k-tile: 6 heads x (64 v-cols + a ones col for the softmax
            # denominator) -> P@V and row-sums come from one M=65 matmul
            v_all = big.tile([128, ST * 390], BF16)  # [seq, t*390 + 65h + d]
            attnT = big.tile([128, N_PAIRS * S], BF16)

            nc.gpsimd.memset(v_all[:], 1.0)
            # causal 128x128 triangle mask (1 where q-col >= k-row), built
            # once: the per-diag-tile masking is a DVE multiply by this tile
            # instead of a gpsimd affine_select -- affine_selects queued
            # BEHIND the previous pair's normalize partition_broadcast on the
            # strict-FIFO gpsimd at every pair boundary, stalling av(j0) ~5us.
            trimask = consts.tile([128, 128], BF16)
            nc.gpsimd.memset(trimask[:], 1.0)
            nc.gpsimd.affine_select(
                out=trimask[:], in_=trimask[:],
                compare_op=mybir.AluOpType.is_ge,
                fill=0.0, base=0, pattern=[[1, 128]], channel_multiplier=-1,
            )
            # CRITICAL lead-in inputs as SINGLE multi-dim strided DMAs (each
            # dma_start costs ~0.6us of ISSUE time on its trigger engine, so
            # issue count is what matters): the w columns the first attention
            # group needs ({0:128 m0-qT, 384:512 m3-kT, 768:896 pair0-v} per
            # k-chunk) in one DMA on sync, and the xT g0 quarter (cols 0:512
            # of every k-chunk) in one DMA on scalar, in parallel.
            # qT/kT weights first ({m0,m3}, in chunk halves so the first
            # matmuls start as soon as chunks 0-2 land), then the pair0-v
            # blocks (not read until av(j=0) at ~19us)
            nc.sync.dma_start(out=w_all[:, 0:768], in_=wqkv_d[:, 0:768])
            nc.sync.dma_start(out=w_all[:, 768:1536], in_=wqkv_d[:, 768:1536])
            nc.sync.dma_start(out=w_all[:, 1536:2304], in_=wqkv_d[:, 1536:2304])
            xT_view_s = xT[:].rearrange("p (k s) -> p k s", k=6)
            nc.scalar.dma_start(out=xT_view_s[:, 0:3, 0:512],
                                in_=xT_d[:, 0:1536])
            nc.scalar.dma_start(out=xT_view_s[:, 3:6, 0:512],
                                in_=xT_d[:, 1536:3072])


            # deferred inputs (one contiguous DMA each), deadline-queued on
            # sync behind the critical lead-in transfers.
            def emit_w_rest():
                nc.sync.dma_start(out=w_all[:, 2304:6912], in_=wqkv_d[:, 2304:6912])

            def emit_xT_quarter(g):
                nc.sync.dma_start(out=xT_view_s[:, :, g * 512:(g + 1) * 512],
                                  in_=xT_d[:, g * 3072:(g + 1) * 3072])

            def emit_wproj():
                nc.sync.dma_start(
                    out=w_proj[:].rearrange("p (c e) -> p c e", c=3),
                    in_=wproj_d[:].rearrange("(c p) e -> p c e", p=128),
                )

            # ---- pools ----
            stps = outer.enter_context(tc.tile_pool(name="stps", bufs=2, space="PSUM"))
            avps = outer.enter_context(tc.tile_pool(name="avps", bufs=2, space="PSUM"))
            auxps = outer.enter_context(tc.tile_pool(name="auxps", bufs=2, space="PSUM"))
            ptp = outer.enter_context(tc.tile_pool(name="ptp", bufs=6))
            avsb = outer.enter_context(tc.tile_pool(name="avsb", bufs=4))
            rcp = outer.enter_context(tc.tile_pool(name="rcp", bufs=6))
            bcp = outer.enter_context(tc.tile_pool(name="bcp", bufs=6))
            shtmp = outer.enter_context(tc.tile_pool(name="shtmp", bufs=3))
            ystage = outer.enter_context(tc.tile_pool(name="ystage", bufs=3))

            v_view = v_all[:].rearrange("p (t h c) -> p t h c", t=ST, h=HG_HEADS)

            # ---- work-unit emitters (each emits a small PE-dense chunk) ----
            def emit_qk_group(m, g):
                # qkT[:, m*S + g*512 : +512] = (W[:, m-block].T @ xT)[:, g-block] + bias
                ps = auxps.tile([128, 512], F32, tag="aux")
                for k in range(6):
                    wc = wcol(k, f"m{m}")
                    nc.tensor.matmul(
                        ps[:],
                        w_all[:, wc:wc + 128],
                        xT[:, k * S + g * 512:k * S + (g + 1) * 512],
                        start=(k == 0), stop=(k == 5),
                    )
                nc.vector.tensor_scalar_add(
                    qkT[:, m * S + g * 512:m * S + (g + 1) * 512],
                    ps[:], bias_qk[:, m:m + 1],
                )

            def emit_v_tile(pair, t):
                # v rows t*128.. for this pair's two heads (N=128); split by
                # pair so each attention slot computes only its own v work
                ps = auxps.tile([128, 128], F32, tag="aux")
                for k in range(6):
                    wc = wcol(k, f"v{pair}")
                    nc.tensor.matmul(
                        ps[:],
                        xT[:, k * S + t * 128:k * S + (t + 1) * 128],
                        w_all[:, wc:wc + 128],
                        start=(k == 0), stop=(skip_vbias and k == 5),
                    )
                if not skip_vbias:
                    nc.tensor.matmul(   # += ones^T[1,128].T @ bias_v[1,128]
                        ps[:], ones_row[:],
                        bias_v[:, pair * 128:(pair + 1) * 128],
                        start=False, stop=True,
                    )
                nc.vector.tensor_copy(
                    v_view[:, t, 2 * pair:2 * pair + 2, 0:64],
                    ps[:].rearrange("p (h d) -> p h d", h=2),
                )

            ys_pending = {}
            drain_ps = {}

            def emit_proj_partial(t, use_avps):
                # drain phase: pairs 0/1 of a proj tile accumulate while the
                # final normalize chain resolves (pair 2 would block the
                # strict-FIFO PE queue).  Tile t8 borrows the now-free AV
                # PSUM banks so two tiles can be in flight alongside the
                # aux pool.
                pool = avps if use_avps else auxps
                tag = "av" if use_avps else "aux"
                psA = pool.tile([128, 512], F32, tag=tag)
                psB = pool.tile([128, 256], F32, tag=tag)
                for p in range(2):
                    lhsT = attnT[:, p * S + t * 128:p * S + (t + 1) * 128]
                    nc.tensor.matmul(psA[:], lhsT, w_proj[:, p * N_EMBD:p * N_EMBD + 512],
                                     start=(p == 0), stop=False)
                    nc.tensor.matmul(psB[:], lhsT,
                                     w_proj[:, p * N_EMBD + 512:(p + 1) * N_EMBD],
                                     start=(p == 0), stop=False)
                drain_ps[t] = (psA, psB)

            def emit_proj_finish(t):
                psA, psB = drain_ps.pop(t)
                lhsT = attnT[:, 2 * S + t * 128:2 * S + (t + 1) * 128]
                nc.tensor.matmul(psA[:], lhsT, w_proj[:, 2 * N_EMBD:2 * N_EMBD + 512],
                                 start=False, stop=True)
                nc.tensor.matmul(psB[:], lhsT,
                                 w_proj[:, 2 * N_EMBD + 512:3 * N_EMBD],
                                 start=False, stop=True)
                ys = ystage.tile([128, 2 * N_EMBD], BF16, tag="ys")
                AFc = mybir.ActivationFunctionType.Copy
                nc.scalar.activation(ys[:, 0:512], psA[:], AFc)
                nc.scalar.activation(ys[:, 512:768], psB[:], AFc)
                nc.sync.dma_start(out=y_d[t * 128:(t + 1) * 128, :],
                                  in_=ys[:, 0:768])

            def emit_proj_tile(t, drain=False):
                # stage into the left/right half of a 2-tile ystage buffer;
                # the odd tile of each pair issues one batched y DMA.  In the
                # post-exp drain the PSUM->stage copies run on the (now idle)
                # ACT engine so they never queue behind DVE normalize work.
                psA = auxps.tile([128, 512], F32, tag="aux")
                psB = auxps.tile([128, 256], F32, tag="aux")
                for p in range(N_PAIRS):
                    lhsT = attnT[:, p * S + t * 128:p * S + (t + 1) * 128]
                    nc.tensor.matmul(psA[:], lhsT, w_proj[:, p * N_EMBD:p * N_EMBD + 512],
                                     start=(p == 0), stop=(p == N_PAIRS - 1))
                    nc.tensor.matmul(psB[:], lhsT,
                                     w_proj[:, p * N_EMBD + 512:(p + 1) * N_EMBD],
                                     start=(p == 0), stop=(p == N_PAIRS - 1))
                if drain:
                    # drain phase: per-tile stores (a 2-tile batch would hold
                    # the last store until both tiles finish) and ACT copies
                    # (the DVE is busy with the final normalize)
                    ys = ystage.tile([128, 2 * N_EMBD], BF16, tag="ys")
                    AFc = mybir.ActivationFunctionType.Copy
                    nc.scalar.activation(ys[:, 0:512], psA[:], AFc)
                    nc.scalar.activation(ys[:, 512:768], psB[:], AFc)
                    nc.sync.dma_start(out=y_d[t * 128:(t + 1) * 128, :],
                                      in_=ys[:, 0:768])
                    return
                if t % 2 == 0:
                    ys = ystage.tile([128, 2 * N_EMBD], BF16, tag="ys")
                    ys_pending[t] = ys
                else:
                    ys = ys_pending.pop(t - 1)
                half = (t % 2) * N_EMBD
                nc.vector.tensor_copy(ys[:, half:half + 512], psA[:])
                nc.vector.tensor_copy(ys[:, half + 512:half + 768], psB[:])
                if t % 2 == 1:
                    b = t // 2
                    nc.sync.dma_start(
                        out=y_d[b * 256:(b + 1) * 256, :]
                            .rearrange("(i p) e -> p i e", p=128),
                        in_=ys[:].rearrange("p (i e) -> p i e", i=2),
                    )

            # ---- deadline-driven background work queue ----
            # Attention groups execute in a fixed order; (pair, g, j) maps to
            # a global step.  Each qkv/proj work unit carries the step by
            # which it MUST be emitted (Tile deps are emission-order-based:
            # a read emitted before its producer gets no dependency).  Units
            # are pulled with LOOKAHEAD steps of slack so the PE always has
            # background matmuls to chew on while ACT runs exp.
            # pair-2 groups run [1,0,3,2]: each group's normalize + proj
            # tiles emit early in the FOLLOWING group (tight deadlines), so
            # after the last exp only group g2's normalize + proj t8-11
            # remain.
            group_order = {0: [0, 1, 2, 3], 1: [0, 1, 2, 3], 2: [1, 0, 3, 2]}
            step_base = {}
            _acc = 0
            for _p in range(N_PAIRS):
                for _g in group_order[_p]:
                    step_base[(_p, _g)] = _acc
                    _acc += 4 * _g + 4
            TOTAL_STEPS = _acc
            LOOKAHEAD = 32

            work_q = []   # sorted list of (deadline_step, seq, fn)
            _seq = [0]

            def push(deadline, fn):
                import bisect
                _seq[0] += 1
                bisect.insort(work_q, (deadline, _seq[0], fn))

            def pull_work(cur_step):
                # overdue units MUST emit now (correctness: emission order
                # defines Tile dependencies); otherwise spread at one unit
                # per step so the background work stays evenly interleaved.
                while work_q and work_q[0][0] <= cur_step:
                    work_q.pop(0)[2]()
                for _ in range(2):
                    if work_q and work_q[0][0] <= cur_step + LOOKAHEAD:
                        work_q.pop(0)[2]()

            # ---- attention group with interleaved background units ----
            sts_all = {}
            pts_all = {}

            def scores_pg(pair, g, j):
                q0 = pair * S
                k0 = (3 + pair) * S
                diag_r = j - 4 * g
                c0 = 128 * diag_r if diag_r >= 0 else 0
                st = stps.tile([128, 1024], F32, tag="st")
                nc.tensor.matmul(
                    st[:, c0:512],
                    qkT[0:64, k0 + j * 128:k0 + (j + 1) * 128],
                    qkT[0:64, q0 + g * 512 + c0:q0 + (g + 1) * 512],
                    start=True, stop=True, tile_position=(0, 0),
                )
                nc.tensor.matmul(
                    st[:, 512 + c0:1024],
                    qkT[64:128, k0 + j * 128:k0 + (j + 1) * 128],
                    qkT[64:128, q0 + g * 512 + c0:q0 + (g + 1) * 512],
                    start=True, stop=True, tile_position=(64, 0),
                )
                sts_all[(pair, g, j)] = (st, c0)

            def expmask_pg(pair, g, j):
                st, c0 = sts_all.pop((pair, g, j))
                pt = ptp.tile([128, 1024], BF16, tag="pt")
                if c0 >= 256:
                    # deep-diagonal tile: head1's [512:512+c0] range is never
                    # read by its AV matmul, so exp the two live ranges
                    # separately (saves c0*128 ACT elements, > the ~170ns
                    # extra instruction cost once c0 >= 256)
                    nc.scalar.activation(pt[:, c0:512], st[:, c0:512],
                                         AF.Exp, bias=0.0, scale=0.125)
                    nc.scalar.activation(pt[:, 512 + c0:1024], st[:, 512 + c0:1024],
                                         AF.Exp, bias=0.0, scale=0.125)
                else:
                    nc.scalar.activation(pt[:, c0:1024], st[:, c0:1024],
                                         AF.Exp, bias=0.0, scale=0.125)
                diag_r = j - 4 * g
                if diag_r >= 0:
                    with nc.allow_low_precision(reason="bf16 causal mask"):
                        for h in range(2):
                            nc.vector.tensor_mul(
                                pt[:, h * 512 + c0:h * 512 + c0 + 128],
                                pt[:, h * 512 + c0:h * 512 + c0 + 128],
                                trimask[:],
                            )
                pts_all[(pair, g, j)] = (pt, c0)

            def emit_attn_group(pair, g, nxt_ctx=None):
                njt = 4 * g + 4
                av0 = avps.tile([65, 512], F32, tag="av")
                av1 = avps.tile([65, 512], F32, tag="av")

                def av(j):
                    pt, c0 = pts_all.pop((pair, g, j))
                    first, last = (j == 0), (j == njt - 1)
                    for h, avt in ((0, av0), (1, av1)):
                        nc.tensor.matmul(
                            avt[0:65, c0:512],
                            v_all[:, j * 390 + (2 * pair + h) * 65:
                                  j * 390 + (2 * pair + h) * 65 + 65],
                            pt[:, h * 512 + c0:(h + 1) * 512],
                            start=first, stop=last,
                        )

                if (pair, g, 0) not in pts_all:
                    scores_pg(pair, g, 0)
                    expmask_pg(pair, g, 0)
                base = step_base[(pair, g)]
                for j in range(njt):
                    if j + 1 < njt:
                        scores_pg(pair, g, j + 1)
                        expmask_pg(pair, g, j + 1)
                    pull_work(base + j)
                    av(j)

                # evacuate the AV accumulators to SBUF (one copy per head into
                # a shared [65,1024] staging tile -- frees the PSUM banks for
                # the next group's AV almost immediately); the
                # recip/redistribute/multiply chain is DEFERRED into the next
                # group's instruction stream so it never stalls the PE at the
                # group boundary.
                avs = avsb.tile([65, 1024], F32, tag="avsb")

                def evacuate():
                    if g == group_order[pair][-1]:
                        # pair boundary: evacuate on ACT (it idles there
                        # while the DVE works the normalize chain)
                        AFc = mybir.ActivationFunctionType.Copy
                        nc.scalar.activation(avs[:, 0:512], av0[:], AFc)
                        nc.scalar.activation(avs[:, 512:1024], av1[:], AFc)
                    else:
                        nc.vector.tensor_copy(avs[:, 0:512], av0[:])
                        nc.vector.tensor_copy(avs[:, 512:1024], av1[:])

                final = (pair == 2 and g == group_order[2][-1])
                if final:
                    evacuate()
                else:
                    # deferred to the NEXT group's step 0: the copies then
                    # queue BEHIND its first exp in the engine FIFO (emitted
                    # inline they delayed that exp ~1.2us at pair
                    # boundaries) but still ahead of its av(0), which needs
                    # these PSUM banks back.
                    push(base + njt, evacuate)

                # with the consolidated input DMAs the sync queue is near
                # idle mid-kernel, so all normalize DMAs ride it (HWDGE; the
                # gpsimd SWDGE path costs ~1us + library reloads and stalled
                # the chain behind affine_selects in practice).
                dma_eng = nc.sync

                def normalize():
                    cols = slice(pair * S + g * 512, pair * S + (g + 1) * 512)
                    # DVE reciprocal runs ~9 cyc/elem PER LANE: on [1,1024]
                    # it would cost ~6us.  Reshape both heads' denominators
                    # to [128,8] via ONE SBUF DMA (flat row-major pairing:
                    # partition p <- cols 8p..8p+7, so p<64 is head0) so the
                    # recip uses 128 lanes (~0.2us), then shape back to
                    # [1,1024] for the gpsimd partition broadcasts.
                    dn8 = rcp.tile([128, 8], F32, tag="dn8")
                    dma_eng.dma_start(out=dn8[:], in_=avs[64:65, :])
                    if final:
                        # head1's UNNORMALIZED rows shifted to partitions
                        # 64-127 in parallel with the reciprocal chain; the
                        # in-place multiply below reads the PE broadcast at
                        # partitions 64-127 directly, cutting the
                        # mult->shift->sem tail (~2.6us measured) off the
                        # final critical path.
                        sh = shtmp.tile([128, 512], F32, tag="sh")
                        dma_eng.dma_start(out=sh[64:128, :],
                                          in_=avs[0:64, 512:1024])
                    with nc.allow_low_precision(reason="softmax normalize bf16"):
                        if final:
                            # drain phase: the PE is idle and the scores PSUM
                            # pool is free, so broadcast the reciprocals with
                            # a rank-1 matmul (trimask row 0 is all-ones)
                            # instead of two serial ~1us partition_broadcasts
                            # -- and head1's multiply reads the PSUM
                            # broadcast directly (every row is identical), so
                            # no gpsimd at all on the final critical chain.
                            rc8b = rcp.tile([128, 8], BF16, tag="rc8b")
                            nc.vector.reciprocal(rc8b[:], dn8[:])
                            rc2b = rcp.tile([1, 1024], BF16, tag="rc2b")
                            dma_eng.dma_start(out=rc2b[:], in_=rc8b[:])
                            bc_ps = stps.tile([128, 1024], F32, tag="st")
                            for h in range(2):
                                nc.tensor.matmul(
                                    bc_ps[:, h * 512:(h + 1) * 512],
                                    trimask[0:1, 0:128],
                                    rc2b[:, h * 512:(h + 1) * 512],
                                    start=True, stop=True,
                                )
                            nc.vector.tensor_mul(attnT[0:64, cols],
                                                 avs[0:64, 0:512],
                                                 bc_ps[0:64, 0:512])
                            nc.vector.tensor_mul(attnT[64:128, cols],
                                                 sh[64:128, :],
                                                 bc_ps[64:128, 512:1024])
                            return
                        rc8 = rcp.tile([128, 8], F32, tag="rc8")
                        nc.vector.reciprocal(rc8[:], dn8[:])
                        rc2 = rcp.tile([1, 1024], F32, tag="rc2")
                        dma_eng.dma_start(out=rc2[:], in_=rc8[:])
                        for h in range(2):
                            bc = bcp.tile([64, 512], F32)
                            nc.gpsimd.partition_broadcast(
                                bc[:], rc2[:, h * 512:(h + 1) * 512], channels=64)
                            if h == 0:
                                nc.vector.tensor_mul(attnT[0:64, cols],
                                                     avs[0:64, 0:512], bc[:])
                            else:
                                # DVE lanes are partition-locked: odd head's
                                # rows 64-127 via an SBUF bounce + DMA shift
                                tmp = shtmp.tile([64, 512], BF16)
                                nc.vector.tensor_mul(tmp[:], avs[0:64, 512:1024],
                                                     bc[:])
                                nc.sync.dma_start(out=attnT[64:128, cols],
                                                  in_=tmp[:])

                nxt = base + njt
                if pair == 2:
                    # tight deadlines: normalize pops at the next group's
                    # step 0 (eligible from nxt+1-LOOKAHEAD, head of queue by
                    # (deadline, seq)), proj tiles follow one per step.  For
                    # the final group nxt == TOTAL_STEPS and these drain
                    # immediately after the j-loop, in push order.
                    push(nxt + 1, normalize)
                    if final:
                        t0 = 4 * g
                        push(nxt + 2, lambda: emit_proj_partial(t0, True))
                        push(nxt + 3, lambda: emit_proj_partial(t0 + 1, False))
                        push(nxt + 4, lambda: emit_proj_finish(t0))
                        push(nxt + 5, lambda: emit_proj_finish(t0 + 1))
                        push(nxt + 6, lambda: emit_proj_tile(t0 + 2, drain=True))
                        push(nxt + 7, lambda: emit_proj_tile(t0 + 3, drain=True))
                    else:
                        for i, t in enumerate(range(4 * g, 4 * g + 4)):
                            push(nxt + 2 + i, lambda t=t: emit_proj_tile(t))
                else:
                    # pairs 0/1: keep the relaxed deadline so the broadcast
                    # queues behind the next group's first affine_selects.
                    push(nxt + LOOKAHEAD, normalize)

            # ================= schedule =================
            # upfront: just enough qkv for attn(0, g0); v t0-3 go through
            # the deadline queue (first read at av(j=t) of group (0,0))
            emit_qk_group(3, 0)          # kT pair 0, seq 0-511
            emit_qk_group(0, 0)          # qT pair 0, seq 0-511

            # deadlines: qT(p, g) is read only by group (p, g); kT(p, g') is
            # read by EVERY group (p, g >= g'), so its deadline is the
            # earliest-executing such group - for pair 2 (non-monotone group
            # order) that is the first group of the pair for ALL kT chunks.
            # qT/kT for pairs 1/2 run THREE steps early: emitted
            # just-in-time (base-1) the next group's first scores wait ~2us
            # for the qk chain + bias add, stalling the exp stream at every
            # group boundary.  Pair 0's stay just-in-time: its xT quarters
            # are still IN FLIGHT, and emitting compute against an un-landed
            # DMA parks a PSUM buffer + the strict-FIFO PE queue on it
            # (measured +40us!).
            for p in range(N_PAIRS):
                for g in range(NG):
                    if (p, g) == (0, 0):
                        continue
                    slack = 3 if p > 0 else 1
                    kt_dl = min(step_base[(p, gg)] for gg in range(g, NG)) - slack
                    push(kt_dl, lambda m=3 + p, g=g: emit_qk_group(m, g))
                    push(step_base[(p, g)] - slack,
                         lambda m=p, g=g: emit_qk_group(m, g))
            # v(pair, t) is first read at av(j=t) of the earliest-executing
            # group g of that pair with 4g+3 >= t
            for p in range(N_PAIRS):
                for t in range(16):
                    dl = min(step_base[(p, g)]
                             for g in group_order[p] if 4 * g + 3 >= t) + t
                    push(dl, lambda p=p, t=t: emit_v_tile(p, t))
            # deferred w complement: needed first by pair-1 qT/kT/v work
            # (earliest deadline around step_base[(1,0)]-3)
            push(step_base[(0, 2)], emit_w_rest)
            # xT quarter g is first read by qk(0, g) units (deadline base-1)
            for g in range(1, NG):
                push(step_base[(0, g)] - 2, lambda g=g: emit_xT_quarter(g))
            # w_proj is first read by proj units in pair 2
            push(step_base[(1, 0)], emit_wproj)

            seq = [(p, g) for p in range(N_PAIRS) for g in group_order[p]]
            for i, (pair, g) in enumerate(seq):
                nxt_ctx = seq[i + 1] if i + 1 < len(seq) else None
                emit_attn_group(pair, g, nxt_ctx)

            # drain in deadline order: the final group's normalize precedes
            # its proj tiles (same-ordered deadlines)
            while work_q:
                work_q.pop(0)[2]()

    nc.compile()
    return nc


def _numpy_fallback(x, mask, W_attn, b_attn, W_proj, b_proj):
    qkv = x @ W_attn + b_attn
    q, k, v = np.split(qkv, 3, axis=-1)

    def heads(t):
        return t.reshape(B, S, N_HEAD, HEAD_DIM).transpose(0, 2, 1, 3)

    q, k, v = heads(q), heads(k), heads(v)
    attn = np.einsum("bhqd,bhkd->bhqk", q, k) / np.sqrt(np.float32(HEAD_DIM))
    attn = attn + mask * (-1e9)
    attn = attn - attn.max(axis=-1, keepdims=True)
    attn = np.exp(attn)
    attn = attn / attn.sum(axis=-1, keepdims=True)
    out = np.einsum("bhqk,bhkd->bhqd", attn, v)
    out = out.transpose(0, 2, 1, 3).reshape(B, S, N_EMBD)
    return (out @ W_proj + b_proj).astype(np.float32)


def _pack_w(Wc):
    """[768, 1152] per-core qkv weight -> [128, 6912] packed layout: cols
    0:2304 = k-major {m0, m3, v0} blocks (the first attention group's
    critical columns), cols 2304:6912 = k-major {m1, m2, m4, m5, v1, v2}."""
    critA = np.concatenate([Wc[:, 0:128], Wc[:, 384:512]], axis=1)  # [768, 256]
    critAP = critA.reshape(6, 128, 256).transpose(1, 0, 2).reshape(128, 1536)
    vblkP = Wc[:, 768:896].reshape(6, 128, 128).transpose(1, 0, 2).reshape(128, 768)
    rest = np.concatenate(
        [Wc[:, 128:384], Wc[:, 512:768], Wc[:, 896:1152]], axis=1)   # [768, 768]
    restP = rest.reshape(6, 128, 768).transpose(1, 0, 2).reshape(128, 4608)
    return np.concatenate([critAP, vblkP, restP], axis=1)


def make_in_maps(x, W_attn, b_attn, W_proj):
    bf16 = ml_dtypes.bfloat16
    in_maps = []
    for c in range(N_CORES):
        b, hg = divmod(c, 2)
        o = HG_DIM * hg
        Wc = np.concatenate(
            [W_attn[:, o:o + HG_DIM],
             W_attn[:, 768 + o:768 + o + HG_DIM],
             W_attn[:, 1536 + o:1536 + o + HG_DIM]], axis=1)
        xTc = x[b].T.astype(bf16)   # [768, 2048]
        xT_packed = (xTc.reshape(6, 128, 4, 512).transpose(1, 2, 0, 3)
                     .reshape(128, 4 * 3072))
        in_maps.append({
            "xT": np.ascontiguousarray(xT_packed),
            "w_qkv": np.ascontiguousarray(_pack_w(Wc).astype(bf16)),
            "b_qk": np.ascontiguousarray(np.concatenate(
                [b_attn[o:o + HG_DIM], b_attn[768 + o:768 + o + HG_DIM]])),
            "b_v": np.ascontiguousarray(b_attn[1536 + o:1536 + o + HG_DIM]).astype(bf16),
            "w_proj": np.ascontiguousarray(W_proj[o:o + HG_DIM, :].astype(bf16)),
            "ones": np.ones((1, 128), dtype=bf16),
        })
    return in_maps


def kernel(x, mask, W_attn, b_attn, W_proj, b_proj):
    global LAST_RESULTS
    x = np.asarray(x, dtype=np.float32)
    mask = np.asarray(mask, dtype=np.float32)
    W_attn = np.asarray(W_attn, dtype=np.float32)
    b_attn = np.asarray(b_attn, dtype=np.float32)
    W_proj = np.asarray(W_proj, dtype=np.float32)
    b_proj = np.asarray(b_proj, dtype=np.float32)

    # the kernel exploits causal structure; verify the mask actually is causal
    causal = 1.0 - np.tril(np.ones((S, S), dtype=np.float32))
    if mask.shape != (1, 1, S, S) or not np.array_equal(mask[0, 0], causal):
        return _numpy_fallback(x, mask, W_attn, b_attn, W_proj, b_proj)

    from concourse.bass_utils import run_bass_kernel_spmd

    skip_vbias = not b_attn[1536:2304].any()   # v-bias exactly zero
    if skip_vbias not in _PROGRAMS:
        _PROGRAMS[skip_vbias] = _build_program(skip_vbias=skip_vbias)

    in_maps = make_in_maps(x, W_attn, b_attn, W_proj)

    trace = bool(int(os.environ.get("ATTN_KERNEL_TRACE", "0")))
    res = run_bass_kernel_spmd(_PROGRAMS[skip_vbias], in_maps,
                               list(range(N_CORES)), trace=trace)
    LAST_RESULTS = res

    y = np.zeros((B, S, N_EMBD), dtype=np.float32)
    for c in range(N_CORES):
        y[c // 2] += res.results[c]["y"].astype(np.float32)
    y += b_proj
    return y


# revision 63
# speedup vs baseline: 1.0226x; 1.0071x over previous
"""Trainium2 Bass kernel for a 12-head causal attention block (GPT-2 style).

Problem: x:[4,2048,768] -> qkv = x@W_attn+b_attn, causal softmax attention
(12 heads, d=64), out @ W_proj + b_proj.

Sharding over 8 NeuronCores: core c handles batch b=c//2 (data parallel) and
head-group hg=c%2 (6 heads = 3 head-pairs, tensor parallel on the qkv
columns / proj rows).  Each core returns a partial projection output; the
host sums the two head-group partials per batch and adds b_proj.

v5 design (~201us, from the 226us v2 baseline; trace-driven changes):
  - final-group normalize: reciprocal broadcast via a rank-1 PE matmul
    (trimask's all-ones row x recips) into a free scores-PSUM tile instead
    of two serial gpsimd partition_broadcasts; head1 is shifted
    UNNORMALIZED in parallel with the chain and multiplied in place at
    partitions 64-127 reading the PSUM broadcast directly (-5us tail).
  - drain proj tiles t8/t9 split into pairs-0/1 accumulation (runs during
    the normalize chain; t8 borrows the freed AV PSUM banks) + pair-2
    finish, so the strict-FIFO PE queue no longer parks on the chain.
  - AV-PSUM evacuation deferred to the NEXT group's first step: emitted
    inline it sat AHEAD of that group's first exp in the engine FIFO
    (+1.2us stall at pair boundaries); deferred it queues behind the exp
    but still before av(0) needs the banks (-3us).
  - critical lead-in loads carry only the m0/m3 qT/kT columns (pair0-v
    deferred; it is not read until ~19us), in chunk halves.
  - deadline-queue tuning: LOOKAHEAD 9 -> 32 and up to TWO background
    units pulled per step (the 1-unit/step spread over-throttled the PE's
    background matmul supply; swept 9/12/16/24/32/48, optimum 32);
    deeper SBUF pools (pt/avsb/rc/bc/ystage) to loosen WAR coupling.
  - exp on deep-diagonal tiles (c0>=256) split into the two live per-head
    ranges (head1's [512:512+c0] was computed but never read).
  - causal masking is a DVE multiply by a precomputed 128x128 triangle
    tile instead of gpsimd affine_select: the affine_selects queued behind
    the previous pair's normalize partition_broadcast on the strict-FIFO
    gpsimd queue, stalling the next pair's first AV ~5-6us at every pair
    boundary (the single biggest win, -9us).
  - inputs are HOST-PACKED so every transfer is a contiguous 2D DMA with
    few issues: xT quarter-major [128, 4*3072], w_qkv split into the
    first-group-critical column blocks {m0-qT, m3-kT, pair0-v} (cols
    0:2304) and the rest; the two critical loads are split in halves on
    the sync+scalar HWDGE rings so the first qk matmul starts as soon as
    the first chunks land.  (The naive [768,2048] xT layout cost a
    768-descriptor strided transfer on the critical path.)
  - normalize: single [65,1024] avsb staging tile for both heads (one
    denominator-reshape DMA, one reciprocal-redistribute DMA, both on
    sync); avsb evacuation runs on ACT for the last group of each pair
    (ACT idles at boundaries, DVE is congested).
  - pair-2 normalize/proj deadlines tightened so each group's proj tiles
    emit during the NEXT group's j-loop, and the drain-phase proj stage
    copies run on ACT; v2 left ~2 groups of proj work after the last exp.
  - qT/kT for pairs 1/2 are emitted 3 steps early (group-boundary exp
    stalls); pair 0's stay just-in-time because its xT quarters are still
    in flight and emitting compute against un-landed DMAs parks PSUM
    buffers and the strict-FIFO PE queue (measured +40us when tried).
  - y partials stored in bf16 (host sums in fp32), batched 2 seq-tiles
    per DMA; ones/bias_v DMAs skipped when the v bias is zero.

  Measured-dead-ends kept out: fp8 (e4m3 per-element quantization error
  does not average out in zero-mean dots -> ~4-6%% output error vs the 2%%
  budget), AV K-split row-tiling (hardware hang), walrus
  --enable-ldw-opt=true (codegen crash), gpsimd SWDGE for bulk input DMAs
  (+40us).
"""

import os
import ml_dtypes
import numpy as np

N_HEAD = 12
N_EMBD = 768
HEAD_DIM = 64
B, S = 4, 2048
N_CORES = 8
HG_HEADS = 6            # heads per core (3 pairs)
HG_DIM = HG_HEADS * HEAD_DIM   # 384
QKV_W = 3 * HG_DIM      # 1152 qkv columns per core
N_PAIRS = 3
ST = S // 128           # 16 seq tiles of 128
NG = S // 512           # 4 seq groups of 512

LAST_RESULTS = None
_PROGRAMS = {}


def _build_program(skip_vbias=False):
    import concourse.bacc as bacc
    import concourse.tile as tile
    from concourse import mybir


    F32 = mybir.dt.float32
    BF16 = mybir.dt.bfloat16
    AF = mybir.ActivationFunctionType

    nc = bacc.Bacc(None, target_bir_lowering=False)
    # host-packed xT, [128, 12288]: col g*3072 + k*512 + s holds
    # xT[k*128+p, g*512+s] -- each 512-seq quarter is one contiguous 2D DMA
    # (the naive [768,2048] layout needed a 768-descriptor strided transfer
    # that sat on the critical path for ~7us).
    xT_d = nc.declare_dram_parameter("xT", [128, 4 * 3072], BF16, isOutput=False)
    # host-packed qkv weights, [128, 6912]: cols 0:2304 hold the
    # first-attention-group-critical blocks {m0-qT, m3-kT, pair0-v} k-major
    # (384 per k-chunk), cols 2304:6912 the complement {m1, m2, m4, m5,
    # v1, v2} k-major (768 per k-chunk) -- so the critical lead-in load and
    # the deferred load are ONE contiguous 2D DMA each.
    wqkv_d = nc.declare_dram_parameter("w_qkv", [128, 54 * 128], BF16, isOutput=False)
    bqk_d = nc.declare_dram_parameter("b_qk", [768], F32, isOutput=False)
    bv_d = nc.declare_dram_parameter("b_v", [HG_DIM], BF16, isOutput=False)
    wproj_d = nc.declare_dram_parameter("w_proj", [HG_DIM, N_EMBD], BF16, isOutput=False)
    ones_d = nc.declare_dram_parameter("ones", [1, 128], BF16, isOutput=False)
    # y partials in bf16: halves the store traffic (the host sums the two
    # head-group partials in fp32; bf16 partial rounding adds ~0.1% error)
    y_d = nc.declare_dram_parameter("y", [S, N_EMBD], BF16, isOutput=True)

    with tile.TileContext(nc) as tc:
        from contextlib import ExitStack

        with ExitStack() as outer:
            consts = outer.enter_context(tc.tile_pool(name="consts", bufs=1))
            ones_row = consts.tile([1, 128], BF16)
            bias_v = consts.tile([1, HG_DIM], BF16)
            if not skip_vbias:
                nc.gpsimd.dma_start(out=ones_row[:], in_=ones_d[:])
                nc.gpsimd.dma_start(
                    out=bias_v[:], in_=bv_d[0:HG_DIM].rearrange("(o v) -> o v", o=1)
                )
            bias_qk = consts.tile([128, 6], F32)      # col m: b_qk[128m:128m+128]
            nc.gpsimd.dma_start(
                out=bias_qk[:], in_=bqk_d[0:768].rearrange("(m p) -> p m", p=128)
            )

            # ---- persistent activations/weights in SBUF (all bf16) ----
            big = outer.enter_context(tc.tile_pool(name="big", bufs=1))
            xT = big.tile([128, 6 * S], BF16)       # [emb-part, k-chunk*2048+seq]
            w_all = big.tile([128, 54 * 128], BF16)  # packed layout (see wqkv_d)

            def wcol(k, which):
                # column of 128-wide weight block `which` of k-chunk k in the
                # packed w_all layout: {m0,m3} k-major (cols 0:1536), then
                # pair0-v blocks (1536:2304), then the rest
                if which == "m0":
                    return k * 256
                if which == "m3":
                    return k * 256 + 128
                if which == "v0":
                    return 1536 + k * 128
                ri = {"m1": 0, "m2": 1, "m4": 2, "m5": 3, "v1": 4, "v2": 5}
                return 2304 + k * 768 + ri[which] * 128
            w_proj = big.tile([128, N_PAIRS * N_EMBD], BF16)
            qkT = big.tile([128, 6 * S], BF16)      # m=0..2 qT pairs, m=3..5 kT pairs
            # per k-tile: 6 heads x (64 v-cols + a ones col for the softmax
            # denominator) -> P@V and row-sums come from one M=65 matmul
            v_all = big.tile([128, ST * 390], BF16)  # [seq, t*390 + 65h + d]
            attnT = big.tile([128, N_PAIRS * S], BF16)

            # scratch operand for the PE warm-up matmuls below
            warm = consts.tile([128, 512], BF16)
            nc.gpsimd.memset(warm[:], 1.0)
            nc.gpsimd.memset(v_all[:], 1.0)
            # causal 128x128 triangle mask (1 where q-col >= k-row), built
            # once: the per-diag-tile masking is a DVE multiply by this tile
            # instead of a gpsimd affine_select -- affine_selects queued
            # BEHIND the previous pair's normalize partition_broadcast on the
            # strict-FIFO gpsimd at every pair boundary, stalling av(j0) ~5us.
            trimask = consts.tile([128, 128], BF16)
            nc.gpsimd.memset(trimask[:], 1.0)
            nc.gpsimd.affine_select(
                out=trimask[:], in_=trimask[:],
                compare_op=mybir.AluOpType.is_ge,
                fill=0.0, base=0, pattern=[[1, 128]], channel_multiplier=-1,
            )
            # CRITICAL lead-in inputs as SINGLE multi-dim strided DMAs (each
            # dma_start costs ~0.6us of ISSUE time on its trigger engine, so
            # issue count is what matters): the w columns the first attention
            # group needs ({0:128 m0-qT, 384:512 m3-kT, 768:896 pair0-v} per
            # k-chunk) in one DMA on sync, and the xT g0 quarter (cols 0:512
            # of every k-chunk) in one DMA on scalar, in parallel.
            # qT/kT weights first ({m0,m3}, in chunk halves so the first
            # matmuls start as soon as chunks 0-2 land), then the pair0-v
            # blocks (not read until av(j=0) at ~19us)
            nc.sync.dma_start(out=w_all[:, 0:768], in_=wqkv_d[:, 0:768])
            nc.sync.dma_start(out=w_all[:, 768:1536], in_=wqkv_d[:, 768:1536])
            nc.sync.dma_start(out=w_all[:, 1536:2304], in_=wqkv_d[:, 1536:2304])
            xT_view_s = xT[:].rearrange("p (k s) -> p k s", k=6)
            nc.scalar.dma_start(out=xT_view_s[:, 0:3, 0:512],
                                in_=xT_d[:, 0:1536])
            nc.scalar.dma_start(out=xT_view_s[:, 3:6, 0:512],
                                in_=xT_d[:, 1536:3072])


            # deferred inputs (one contiguous DMA each), deadline-queued on
            # sync behind the critical lead-in transfers.
            def emit_w_rest():
                nc.sync.dma_start(out=w_all[:, 2304:6912], in_=wqkv_d[:, 2304:6912])

            def emit_xT_quarter(g):
                nc.sync.dma_start(out=xT_view_s[:, :, g * 512:(g + 1) * 512],
                                  in_=xT_d[:, g * 3072:(g + 1) * 3072])

            def emit_wproj():
                nc.sync.dma_start(
                    out=w_proj[:].rearrange("p (c e) -> p c e", c=3),
                    in_=wproj_d[:].rearrange("(c p) e -> p c e", p=128),
                )

            # ---- pools ----
            stps = outer.enter_context(tc.tile_pool(name="stps", bufs=2, space="PSUM"))
            avps = outer.enter_context(tc.tile_pool(name="avps", bufs=2, space="PSUM"))
            auxps = outer.enter_context(tc.tile_pool(name="auxps", bufs=2, space="PSUM"))
            ptp = outer.enter_context(tc.tile_pool(name="ptp", bufs=6))
            avsb = outer.enter_context(tc.tile_pool(name="avsb", bufs=4))
            rcp = outer.enter_context(tc.tile_pool(name="rcp", bufs=6))
            bcp = outer.enter_context(tc.tile_pool(name="bcp", bufs=6))
            shtmp = outer.enter_context(tc.tile_pool(name="shtmp", bufs=3))
            ystage = outer.enter_context(tc.tile_pool(name="ystage", bufs=3))

            # PE clock warm-up: the HAM gate holds the PE at 1.2 GHz until
            # ~3.4us of sustained activity, and the PE is idle from the
            # preamble until the critical DMAs land (~13.9us).  Ten dummy
            # matmuls (run 9.5-13.8us, traced) warm the clock so the real
            # qk chains run at 2.4 GHz (216ns vs 426ns per matmul).
            wps = auxps.tile([128, 512], F32, tag="aux")
            for _ in range(10):
                nc.tensor.matmul(wps[:], warm[:, 0:128], warm[:],
                                 start=True, stop=True)

            v_view = v_all[:].rearrange("p (t h c) -> p t h c", t=ST, h=HG_HEADS)

            # ---- work-unit emitters (each emits a small PE-dense chunk) ----
            def emit_qk_group(m, g):
                # qkT[:, m*S + g*512 : +512] = (W[:, m-block].T @ xT)[:, g-block] + bias
                ps = auxps.tile([128, 512], F32, tag="aux")
                for k in range(6):
                    wc = wcol(k, f"m{m}")
                    nc.tensor.matmul(
                        ps[:],
                        w_all[:, wc:wc + 128],
                        xT[:, k * S + g * 512:k * S + (g + 1) * 512],
                        start=(k == 0), stop=(k == 5),
                    )
                nc.vector.tensor_scalar_add(
                    qkT[:, m * S + g * 512:m * S + (g + 1) * 512],
                    ps[:], bias_qk[:, m:m + 1],
                )

            def emit_v_tile(pair, t):
                # v rows t*128.. for this pair's two heads (N=128); split by
                # pair so each attention slot computes only its own v work
                ps = auxps.tile([128, 128], F32, tag="aux")
                for k in range(6):
                    wc = wcol(k, f"v{pair}")
                    nc.tensor.matmul(
                        ps[:],
                        xT[:, k * S + t * 128:k * S + (t + 1) * 128],
                        w_all[:, wc:wc + 128],
                        start=(k == 0), stop=(skip_vbias and k == 5),
                    )
                if not skip_vbias:
                    nc.tensor.matmul(   # += ones^T[1,128].T @ bias_v[1,128]
                        ps[:], ones_row[:],
                        bias_v[:, pair * 128:(pair + 1) * 128],
                        start=False, stop=True,
                    )
                nc.vector.tensor_copy(
                    v_view[:, t, 2 * pair:2 * pair + 2, 0:64],
                    ps[:].rearrange("p (h d) -> p h d", h=2),
                )

            ys_pending = {}
            drain_ps = {}

            def emit_proj_partial(t, use_avps):
                # drain phase: pairs 0/1 of a proj tile accumulate while the
                # final normalize chain resolves (pair 2 would block the
                # strict-FIFO PE queue).  Tile t8 borrows the now-free AV
                # PSUM banks so two tiles can be in flight alongside the
                # aux pool.
                pool = avps if use_avps else auxps
                tag = "av" if use_avps else "aux"
                psA = pool.tile([128, 512], F32, tag=tag)
                psB = pool.tile([128, 256], F32, tag=tag)
                for p in range(2):
                    lhsT = attnT[:, p * S + t * 128:p * S + (t + 1) * 128]
                    nc.tensor.matmul(psA[:], lhsT, w_proj[:, p * N_EMBD:p * N_EMBD + 512],
                                     start=(p == 0), stop=False)
                    nc.tensor.matmul(psB[:], lhsT,
                                     w_proj[:, p * N_EMBD + 512:(p + 1) * N_EMBD],
                                     start=(p == 0), stop=False)
                drain_ps[t] = (psA, psB)

            def emit_proj_finish(t):
                psA, psB = drain_ps.pop(t)
                lhsT = attnT[:, 2 * S + t * 128:2 * S + (t + 1) * 128]
                nc.tensor.matmul(psA[:], lhsT, w_proj[:, 2 * N_EMBD:2 * N_EMBD + 512],
                                 start=False, stop=True)
                nc.tensor.matmul(psB[:], lhsT,
                                 w_proj[:, 2 * N_EMBD + 512:3 * N_EMBD],
                                 start=False, stop=True)
                ys = ystage.tile([128, 2 * N_EMBD], BF16, tag="ys")
                AFc = mybir.ActivationFunctionType.Copy
                nc.scalar.activation(ys[:, 0:512], psA[:], AFc)
                nc.scalar.activation(ys[:, 512:768], psB[:], AFc)
                nc.sync.dma_start(out=y_d[t * 128:(t + 1) * 128, :],
                                  in_=ys[:, 0:768])

            def emit_proj_tile(t, drain=False):
                # stage into the left/right half of a 2-tile ystage buffer;
                # the odd tile of each pair issues one batched y DMA.  In the
                # post-exp drain the PSUM->stage copies run on the (now idle)
                # ACT engine so they never queue behind DVE normalize work.
                psA = auxps.tile([128, 512], F32, tag="aux")
                psB = auxps.tile([128, 256], F32, tag="aux")
                for p in range(N_PAIRS):
                    lhsT = attnT[:, p * S + t * 128:p * S + (t + 1) * 128]
                    nc.tensor.matmul(psA[:], lhsT, w_proj[:, p * N_EMBD:p * N_EMBD + 512],
                                     start=(p == 0), stop=(p == N_PAIRS - 1))
                    nc.tensor.matmul(psB[:], lhsT,
                                     w_proj[:, p * N_EMBD + 512:(p + 1) * N_EMBD],
                                     start=(p == 0), stop=(p == N_PAIRS - 1))
                if drain:
                    # drain phase: per-tile stores (a 2-tile batch would hold
                    # the last store until both tiles finish) and ACT copies
                    # (the DVE is busy with the final normalize)
                    ys = ystage.tile([128, 2 * N_EMBD], BF16, tag="ys")
                    AFc = mybir.ActivationFunctionType.Copy
                    nc.scalar.activation(ys[:, 0:512], psA[:], AFc)
                    nc.scalar.activation(ys[:, 512:768], psB[:], AFc)
                    nc.sync.dma_start(out=y_d[t * 128:(t + 1) * 128, :],
                                      in_=ys[:, 0:768])
                    return
                if t % 2 == 0:
                    ys = ystage.tile([128, 2 * N_EMBD], BF16, tag="ys")
                    ys_pending[t] = ys
                else:
                    ys = ys_pending.pop(t - 1)
                half = (t % 2) * N_EMBD
                nc.vector.tensor_copy(ys[:, half:half + 512], psA[:])
                nc.vector.tensor_copy(ys[:, half + 512:half + 768], psB[:])
                if t % 2 == 1:
                    b = t // 2
                    nc.sync.dma_start(
                        out=y_d[b * 256:(b + 1) * 256, :]
                            .rearrange("(i p) e -> p i e", p=128),
                        in_=ys[:].rearrange("p (i e) -> p i e", i=2),
                    )

            # ---- deadline-driven background work queue ----
            # Attention groups execute in a fixed order; (pair, g, j) maps to
            # a global step.  Each qkv/proj work unit carries the step by
            # which it MUST be emitted (Tile deps are emission-order-based:
            # a read emitted before its producer gets no dependency).  Units
            # are pulled with LOOKAHEAD steps of slack so the PE always has
            # background matmuls to chew on while ACT runs exp.
            # pair-2 groups run [1,0,3,2]: each group's normalize + proj
            # tiles emit early in the FOLLOWING group (tight deadlines), so
            # after the last exp only group g2's normalize + proj t8-11
            # remain.
            group_order = {0: [0, 1, 2, 3], 1: [0, 1, 2, 3], 2: [1, 0, 3, 2]}
            step_base = {}
            _acc = 0
            for _p in range(N_PAIRS):
                for _g in group_order[_p]:
                    step_base[(_p, _g)] = _acc
                    _acc += 4 * _g + 4
            TOTAL_STEPS = _acc
            LOOKAHEAD = 32

            work_q = []   # sorted list of (deadline_step, seq, fn)
            _seq = [0]

            def push(deadline, fn):
                import bisect
                _seq[0] += 1
                bisect.insort(work_q, (deadline, _seq[0], fn))

            def pull_work(cur_step):
                # overdue units MUST emit now (correctness: emission order
                # defines Tile dependencies); otherwise spread at one unit
                # per step so the background work stays evenly interleaved.
                while work_q and work_q[0][0] <= cur_step:
                    work_q.pop(0)[2]()
                for _ in range(2):
                    if work_q and work_q[0][0] <= cur_step + LOOKAHEAD:
                        work_q.pop(0)[2]()

            # ---- attention group with interleaved background units ----
            sts_all = {}
            pts_all = {}

            def scores_pg(pair, g, j):
                q0 = pair * S
                k0 = (3 + pair) * S
                diag_r = j - 4 * g
                c0 = 128 * diag_r if diag_r >= 0 else 0
                st = stps.tile([128, 1024], F32, tag="st")
                nc.tensor.matmul(
                    st[:, c0:512],
                    qkT[0:64, k0 + j * 128:k0 + (j + 1) * 128],
                    qkT[0:64, q0 + g * 512 + c0:q0 + (g + 1) * 512],
                    start=True, stop=True, tile_position=(0, 0),
                )
                nc.tensor.matmul(
                    st[:, 512 + c0:1024],
                    qkT[64:128, k0 + j * 128:k0 + (j + 1) * 128],
                    qkT[64:128, q0 + g * 512 + c0:q0 + (g + 1) * 512],
                    start=True, stop=True, tile_position=(64, 0),
                )
                sts_all[(pair, g, j)] = (st, c0)

            def expmask_pg(pair, g, j):
                st, c0 = sts_all.pop((pair, g, j))
                pt = ptp.tile([128, 1024], BF16, tag="pt")
                if c0 >= 256:
                    # deep-diagonal tile: head1's [512:512+c0] range is never
                    # read by its AV matmul, so exp the two live ranges
                    # separately (saves c0*128 ACT elements, > the ~170ns
                    # extra instruction cost once c0 >= 256)
                    nc.scalar.activation(pt[:, c0:512], st[:, c0:512],
                                         AF.Exp, bias=0.0, scale=0.125)
                    nc.scalar.activation(pt[:, 512 + c0:1024], st[:, 512 + c0:1024],
                                         AF.Exp, bias=0.0, scale=0.125)
                else:
                    nc.scalar.activation(pt[:, c0:1024], st[:, c0:1024],
                                         AF.Exp, bias=0.0, scale=0.125)
                diag_r = j - 4 * g
                if diag_r >= 0:
                    with nc.allow_low_precision(reason="bf16 causal mask"):
                        for h in range(2):
                            nc.vector.tensor_mul(
                                pt[:, h * 512 + c0:h * 512 + c0 + 128],
                                pt[:, h * 512 + c0:h * 512 + c0 + 128],
                                trimask[:],
                            )
                pts_all[(pair, g, j)] = (pt, c0)

            def emit_attn_group(pair, g, nxt_ctx=None):
                njt = 4 * g + 4
                av0 = avps.tile([65, 512], F32, tag="av")
                av1 = avps.tile([65, 512], F32, tag="av")

                def av(j):
                    pt, c0 = pts_all.pop((pair, g, j))
                    first, last = (j == 0), (j == njt - 1)
                    for h, avt in ((0, av0), (1, av1)):
                        nc.tensor.matmul(
                            avt[0:65, c0:512],
                            v_all[:, j * 390 + (2 * pair + h) * 65:
                                  j * 390 + (2 * pair + h) * 65 + 65],
                            pt[:, h * 512 + c0:(h + 1) * 512],
                            start=first, stop=last,
                        )

                if (pair, g, 0) not in pts_all:
                    scores_pg(pair, g, 0)
                    expmask_pg(pair, g, 0)
                base = step_base[(pair, g)]
                for j in range(njt):
                    if j + 1 < njt:
                        scores_pg(pair, g, j + 1)
                        expmask_pg(pair, g, j + 1)
                    pull_work(base + j)
                    av(j)

                # evacuate the AV accumulators to SBUF (one copy per head into
                # a shared [65,1024] staging tile -- frees the PSUM banks for
                # the next group's AV almost immediately); the
                # recip/redistribute/multiply chain is DEFERRED into the next
                # group's instruction stream so it never stalls the PE at the
                # group boundary.
                avs = avsb.tile([65, 1024], F32, tag="avsb")

                def evacuate():
                    if g == group_order[pair][-1]:
                        # pair boundary: evacuate on ACT (it idles there
                        # while the DVE works the normalize chain)
                        AFc = mybir.ActivationFunctionType.Copy
                        nc.scalar.activation(avs[:, 0:512], av0[:], AFc)
                        nc.scalar.activation(avs[:, 512:1024], av1[:], AFc)
                    else:
                        nc.vector.tensor_copy(avs[:, 0:512], av0[:])
                        nc.vector.tensor_copy(avs[:, 512:1024], av1[:])

                final = (pair == 2 and g == group_order[2][-1])
                if final:
                    evacuate()
                else:
                    # deferred to the NEXT group's step 0: the copies then
                    # queue BEHIND its first exp in the engine FIFO (emitted
                    # inline they delayed that exp ~1.2us at pair
                    # boundaries) but still ahead of its av(0), which needs
                    # these PSUM banks back.
                    push(base + njt, evacuate)

                # with the consolidated input DMAs the sync queue is near
                # idle mid-kernel, so all normalize DMAs ride it (HWDGE; the
                # gpsimd SWDGE path costs ~1us + library reloads and stalled
                # the chain behind affine_selects in practice).
                dma_eng = nc.sync

                def normalize():
                    cols = slice(pair * S + g * 512, pair * S + (g + 1) * 512)
                    # DVE reciprocal runs ~9 cyc/elem PER LANE: on [1,1024]
                    # it would cost ~6us.  Reshape both heads' denominators
                    # to [128,8] via ONE SBUF DMA (flat row-major pairing:
                    # partition p <- cols 8p..8p+7, so p<64 is head0) so the
                    # recip uses 128 lanes (~0.2us), then shape back to
                    # [1,1024] for the gpsimd partition broadcasts.
                    dn8 = rcp.tile([128, 8], F32, tag="dn8")
                    dma_eng.dma_start(out=dn8[:], in_=avs[64:65, :])
                    if final:
                        # head1's UNNORMALIZED rows shifted to partitions
                        # 64-127 in parallel with the reciprocal chain; the
                        # in-place multiply below reads the PE broadcast at
                        # partitions 64-127 directly, cutting the
                        # mult->shift->sem tail (~2.6us measured) off the
                        # final critical path.
                        sh = shtmp.tile([128, 512], F32, tag="sh")
                        dma_eng.dma_start(out=sh[64:128, :],
                                          in_=avs[0:64, 512:1024])
                    with nc.allow_low_precision(reason="softmax normalize bf16"):
                        if final:
                            # drain phase: the PE is idle and the scores PSUM
                            # pool is free, so broadcast the reciprocals with
                            # a rank-1 matmul (trimask row 0 is all-ones)
                            # instead of two serial ~1us partition_broadcasts
                            # -- and head1's multiply reads the PSUM
                            # broadcast directly (every row is identical), so
                            # no gpsimd at all on the final critical chain.
                            rc8b = rcp.tile([128, 8], BF16, tag="rc8b")
                            nc.vector.reciprocal(rc8b[:], dn8[:])
                            rc2b = rcp.tile([1, 1024], BF16, tag="rc2b")
                            dma_eng.dma_start(out=rc2b[:], in_=rc8b[:])
                            bc_ps = stps.tile([128, 1024], F32, tag="st")
                            for h in range(2):
                                nc.tensor.matmul(
                                    bc_ps[:, h * 512:(h + 1) * 512],
                                    trimask[0:1, 0:128],
                                    rc2b[:, h * 512:(h + 1) * 512],
                                    start=True, stop=True,
                                )
                            nc.vector.tensor_mul(attnT[0:64, cols],
                                                 avs[0:64, 0:512],
                                                 bc_ps[0:64, 0:512])
                            nc.vector.tensor_mul(attnT[64:128, cols],
                                                 sh[64:128, :],
                                                 bc_ps[64:128, 512:1024])
                            return
                        rc8 = rcp.tile([128, 8], F32, tag="rc8")
                        nc.vector.reciprocal(rc8[:], dn8[:])
                        rc2 = rcp.tile([1, 1024], F32, tag="rc2")
                        dma_eng.dma_start(out=rc2[:], in_=rc8[:])
                        for h in range(2):
                            bc = bcp.tile([64, 512], F32)
                            nc.gpsimd.partition_broadcast(
                                bc[:], rc2[:, h * 512:(h + 1) * 512], channels=64)
                            if h == 0:
                                nc.vector.tensor_mul(attnT[0:64, cols],
                                                     avs[0:64, 0:512], bc[:])
                            else:
                                # DVE lanes are partition-locked: odd head's
                                # rows 64-127 via an SBUF bounce + DMA shift
                                tmp = shtmp.tile([64, 512], BF16)
                                nc.vector.tensor_mul(tmp[:], avs[0:64, 512:1024],
                                                     bc[:])
                                nc.sync.dma_start(out=attnT[64:128, cols],
                                                  in_=tmp[:])

                nxt = base + njt
                if pair == 2:
                    # tight deadlines: normalize pops at the next group's
                    # step 0 (eligible from nxt+1-LOOKAHEAD, head of queue by
                    # (deadline, seq)), proj tiles follow one per step.  For
                    # the final group nxt == TOTAL_STEPS and these drain
                    # immediately after the j-loop, in push order.
                    push(nxt + 1, normalize)
                    if final:
                        t0 = 4 * g
                        push(nxt + 2, lambda: emit_proj_partial(t0, True))
                        push(nxt + 3, lambda: emit_proj_partial(t0 + 1, False))
                        push(nxt + 4, lambda: emit_proj_finish(t0))
                        push(nxt + 5, lambda: emit_proj_finish(t0 + 1))
                        push(nxt + 6, lambda: emit_proj_tile(t0 + 2, drain=True))
                        push(nxt + 7, lambda: emit_proj_tile(t0 + 3, drain=True))
                    else:
                        for i, t in enumerate(range(4 * g, 4 * g + 4)):
                            push(nxt + 2 + i, lambda t=t: emit_proj_tile(t))
                else:
                    # pairs 0/1: keep the relaxed deadline so the broadcast
                    # queues behind the next group's first affine_selects.
                    push(nxt + LOOKAHEAD, normalize)

            # ================= schedule =================
            # upfront: just enough qkv for attn(0, g0); v t0-3 go through
            # the deadline queue (first read at av(j=t) of group (0,0))
            emit_qk_group(3, 0)          # kT pair 0, seq 0-511
            emit_qk_group(0, 0)          # qT pair 0, seq 0-511

            # deadlines: qT(p, g) is read only by group (p, g); kT(p, g') is
            # read by EVERY group (p, g >= g'), so its deadline is the
            # earliest-executing such group - for pair 2 (non-monotone group
            # order) that is the first group of the pair for ALL kT chunks.
            # qT/kT for pairs 1/2 run THREE steps early: emitted
            # just-in-time (base-1) the next group's first scores wait ~2us
            # for the qk chain + bias add, stalling the exp stream at every
            # group boundary.  Pair 0's stay just-in-time: its xT quarters
            # are still IN FLIGHT, and emitting compute against an un-landed
            # DMA parks a PSUM buffer + the strict-FIFO PE queue on it
            # (measured +40us!).
            for p in range(N_PAIRS):
                for g in range(NG):
                    if (p, g) == (0, 0):
                        continue
                    slack = 3 if p > 0 else 1
                    kt_dl = min(step_base[(p, gg)] for gg in range(g, NG)) - slack
                    push(kt_dl, lambda m=3 + p, g=g: emit_qk_group(m, g))
                    push(step_base[(p, g)] - slack,
                         lambda m=p, g=g: emit_qk_group(m, g))
            # v(pair, t) is first read at av(j=t) of the earliest-executing
            # group g of that pair with 4g+3 >= t
            for p in range(N_PAIRS):
                for t in range(16):
                    dl = min(step_base[(p, g)]
                             for g in group_order[p] if 4 * g + 3 >= t) + t
                    push(dl, lambda p=p, t=t: emit_v_tile(p, t))
            # deferred w complement: needed first by pair-1 qT/kT/v work
            # (earliest deadline around step_base[(1,0)]-3)
            push(step_base[(0, 2)], emit_w_rest)
            # xT quarter g is first read by qk(0, g) units (deadline base-1)
            for g in range(1, NG):
                push(step_base[(0, g)] - 2, lambda g=g: emit_xT_quarter(g))
            # w_proj is first read by proj units in pair 2
            push(step_base[(1, 0)], emit_wproj)

            seq = [(p, g) for p in range(N_PAIRS) for g in group_order[p]]
            for i, (pair, g) in enumerate(seq):
                nxt_ctx = seq[i + 1] if i + 1 < len(seq) else None
                emit_attn_group(pair, g, nxt_ctx)

            # drain in deadline order: the final group's normalize precedes
            # its proj tiles (same-ordered deadlines)
            while work_q:
                work_q.pop(0)[2]()

    nc.compile()
    return nc


def _numpy_fallback(x, mask, W_attn, b_attn, W_proj, b_proj):
    qkv = x @ W_attn + b_attn
    q, k, v = np.split(qkv, 3, axis=-1)

    def heads(t):
        return t.reshape(B, S, N_HEAD, HEAD_DIM).transpose(0, 2, 1, 3)

    q, k, v = heads(q), heads(k), heads(v)
    attn = np.einsum("bhqd,bhkd->bhqk", q, k) / np.sqrt(np.float32(HEAD_DIM))
    attn = attn + mask * (-1e9)
    attn = attn - attn.max(axis=-1, keepdims=True)
    attn = np.exp(attn)
    attn = attn / attn.sum(axis=-1, keepdims=True)
    out = np.einsum("bhqk,bhkd->bhqd", attn, v)
    out = out.transpose(0, 2, 1, 3).reshape(B, S, N_EMBD)
    return (out @ W_proj + b_proj).astype(np.float32)


def _pack_w(Wc):
    """[768, 1152] per-core qkv weight -> [128, 6912] packed layout: cols
    0:2304 = k-major {m0, m3, v0} blocks (the first attention group's
    critical columns), cols 2304:6912 = k-major {m1, m2, m4, m5, v1, v2}."""
    critA = np.concatenate([Wc[:, 0:128], Wc[:, 384:512]], axis=1)  # [768, 256]
    critAP = critA.reshape(6, 128, 256).transpose(1, 0, 2).reshape(128, 1536)
    vblkP = Wc[:, 768:896].reshape(6, 128, 128).transpose(1, 0, 2).reshape(128, 768)
    rest = np.concatenate(
        [Wc[:, 128:384], Wc[:, 512:768], Wc[:, 896:1152]], axis=1)   # [768, 768]
    restP = rest.reshape(6, 128, 768).transpose(1, 0, 2).reshape(128, 4608)
    return np.concatenate([critAP, vblkP, restP], axis=1)


def make_in_maps(x, W_attn, b_attn, W_proj):
    bf16 = ml_dtypes.bfloat16
    in_maps = []
    for c in range(N_CORES):
        b, hg = divmod(c, 2)
        o = HG_DIM * hg
        Wc = np.concatenate(
            [W_attn[:, o:o + HG_DIM],
             W_attn[:, 768 + o:768 + o + HG_DIM],
             W_attn[:, 1536 + o:1536 + o + HG_DIM]], axis=1)
        xTc = x[b].T.astype(bf16)   # [768, 2048]
        xT_packed = (xTc.reshape(6, 128, 4, 512).transpose(1, 2, 0, 3)
                     .reshape(128, 4 * 3072))
        in_maps.append({
            "xT": np.ascontiguousarray(xT_packed),
            "w_qkv": np.ascontiguousarray(_pack_w(Wc).astype(bf16)),
            "b_qk": np.ascontiguousarray(np.concatenate(
                [b_attn[o:o + HG_DIM], b_attn[768 + o:768 + o + HG_DIM]])),
            "b_v": np.ascontiguousarray(b_attn[1536 + o:1536 + o + HG_DIM]).astype(bf16),
            "w_proj": np.ascontiguousarray(W_proj[o:o + HG_DIM, :].astype(bf16)),
            "ones": np.ones((1, 128), dtype=bf16),
        })
    return in_maps


def kernel(x, mask, W_attn, b_attn, W_proj, b_proj):
    global LAST_RESULTS
    x = np.asarray(x, dtype=np.float32)
    mask = np.asarray(mask, dtype=np.float32)
    W_attn = np.asarray(W_attn, dtype=np.float32)
    b_attn = np.asarray(b_attn, dtype=np.float32)
    W_proj = np.asarray(W_proj, dtype=np.float32)
    b_proj = np.asarray(b_proj, dtype=np.float32)

    # the kernel exploits causal structure; verify the mask actually is causal
    causal = 1.0 - np.tril(np.ones((S, S), dtype=np.float32))
    if mask.shape != (1, 1, S, S) or not np.array_equal(mask[0, 0], causal):
        return _numpy_fallback(x, mask, W_attn, b_attn, W_proj, b_proj)

    from concourse.bass_utils import run_bass_kernel_spmd

    skip_vbias = not b_attn[1536:2304].any()   # v-bias exactly zero
    if skip_vbias not in _PROGRAMS:
        _PROGRAMS[skip_vbias] = _build_program(skip_vbias=skip_vbias)

    in_maps = make_in_maps(x, W_attn, b_attn, W_proj)

    trace = bool(int(os.environ.get("ATTN_KERNEL_TRACE", "0")))
    res = run_bass_kernel_spmd(_PROGRAMS[skip_vbias], in_maps,
                               list(range(N_CORES)), trace=trace)
    LAST_RESULTS = res

    y = np.zeros((B, S, N_EMBD), dtype=np.float32)
    for c in range(N_CORES):
        y[c // 2] += res.results[c]["y"].astype(np.float32)
    y += b_proj
    return y


# revision 64
# speedup vs baseline: 1.0288x; 1.0061x over previous
"""Trainium2 Bass kernel for a 12-head causal attention block (GPT-2 style).

Problem: x:[4,2048,768] -> qkv = x@W_attn+b_attn, causal softmax attention
(12 heads, d=64), out @ W_proj + b_proj.

Sharding over 8 NeuronCores: core c handles batch b=c//2 (data parallel) and
head-group hg=c%2 (6 heads = 3 head-pairs, tensor parallel on the qkv
columns / proj rows).  Each core returns a partial projection output; the
host sums the two head-group partials per batch and adds b_proj.

v5 design (~201us, from the 226us v2 baseline; trace-driven changes):
  - final-group normalize: reciprocal broadcast via a rank-1 PE matmul
    (trimask's all-ones row x recips) into a free scores-PSUM tile instead
    of two serial gpsimd partition_broadcasts; head1 is shifted
    UNNORMALIZED in parallel with the chain and multiplied in place at
    partitions 64-127 reading the PSUM broadcast directly (-5us tail).
  - drain proj tiles t8/t9 split into pairs-0/1 accumulation (runs during
    the normalize chain; t8 borrows the freed AV PSUM banks) + pair-2
    finish, so the strict-FIFO PE queue no longer parks on the chain.
  - AV-PSUM evacuation deferred to the NEXT group's first step: emitted
    inline it sat AHEAD of that group's first exp in the engine FIFO
    (+1.2us stall at pair boundaries); deferred it queues behind the exp
    but still before av(0) needs the banks (-3us).
  - critical lead-in loads carry only the m0/m3 qT/kT columns (pair0-v
    deferred; it is not read until ~19us), in chunk halves.
  - deadline-queue tuning: LOOKAHEAD 9 -> 32 and up to TWO background
    units pulled per step (the 1-unit/step spread over-throttled the PE's
    background matmul supply; swept 9/12/16/24/32/48, optimum 32);
    deeper SBUF pools (pt/avsb/rc/bc/ystage) to loosen WAR coupling.
  - exp on deep-diagonal tiles (c0>=256) split into the two live per-head
    ranges (head1's [512:512+c0] was computed but never read).
  - causal masking is a DVE multiply by a precomputed 128x128 triangle
    tile instead of gpsimd affine_select: the affine_selects queued behind
    the previous pair's normalize partition_broadcast on the strict-FIFO
    gpsimd queue, stalling the next pair's first AV ~5-6us at every pair
    boundary (the single biggest win, -9us).
  - inputs are HOST-PACKED so every transfer is a contiguous 2D DMA with
    few issues: xT quarter-major [128, 4*3072], w_qkv split into the
    first-group-critical column blocks {m0-qT, m3-kT, pair0-v} (cols
    0:2304) and the rest; the two critical loads are split in halves on
    the sync+scalar HWDGE rings so the first qk matmul starts as soon as
    the first chunks land.  (The naive [768,2048] xT layout cost a
    768-descriptor strided transfer on the critical path.)
  - normalize: single [65,1024] avsb staging tile for both heads (one
    denominator-reshape DMA, one reciprocal-redistribute DMA, both on
    sync); avsb evacuation runs on ACT for the last group of each pair
    (ACT idles at boundaries, DVE is congested).
  - pair-2 normalize/proj deadlines tightened so each group's proj tiles
    emit during the NEXT group's j-loop, and the drain-phase proj stage
    copies run on ACT; v2 left ~2 groups of proj work after the last exp.
  - qT/kT for pairs 1/2 are emitted 3 steps early (group-boundary exp
    stalls); pair 0's stay just-in-time because its xT quarters are still
    in flight and emitting compute against un-landed DMAs parks PSUM
    buffers and the strict-FIFO PE queue (measured +40us when tried).
  - y partials stored in bf16 (host sums in fp32), batched 2 seq-tiles
    per DMA; ones/bias_v DMAs skipped when the v bias is zero.

  Measured-dead-ends kept out: fp8 (e4m3 per-element quantization error
  does not average out in zero-mean dots -> ~4-6%% output error vs the 2%%
  budget), AV K-split row-tiling (hardware hang), walrus
  --enable-ldw-opt=true (codegen crash), gpsimd SWDGE for bulk input DMAs
  (+40us).
"""

import os
import ml_dtypes
import numpy as np

N_HEAD = 12
N_EMBD = 768
HEAD_DIM = 64
B, S = 4, 2048
N_CORES = 8
HG_HEADS = 6            # heads per core (3 pairs)
HG_DIM = HG_HEADS * HEAD_DIM   # 384
QKV_W = 3 * HG_DIM      # 1152 qkv columns per core
N_PAIRS = 3
ST = S // 128           # 16 seq tiles of 128
NG = S // 512           # 4 seq groups of 512

LAST_RESULTS = None
_PROGRAMS = {}


def _build_program(skip_vbias=False):
    import concourse.bacc as bacc
    import concourse.tile as tile
    from concourse import mybir


    F32 = mybir.dt.float32
    BF16 = mybir.dt.bfloat16
    AF = mybir.ActivationFunctionType

    nc = bacc.Bacc(None, target_bir_lowering=False)
    # host-packed xT, [128, 12288]: col g*3072 + k*512 + s holds
    # xT[k*128+p, g*512+s] -- each 512-seq quarter is one contiguous 2D DMA
    # (the naive [768,2048] layout needed a 768-descriptor strided transfer
    # that sat on the critical path for ~7us).
    xT_d = nc.declare_dram_parameter("xT", [128, 4 * 3072], BF16, isOutput=False)
    # host-packed qkv weights, [128, 6912]: cols 0:2304 hold the
    # first-attention-group-critical blocks {m0-qT, m3-kT, pair0-v} k-major
    # (384 per k-chunk), cols 2304:6912 the complement {m1, m2, m4, m5,
    # v1, v2} k-major (768 per k-chunk) -- so the critical lead-in load and
    # the deferred load are ONE contiguous 2D DMA each.
    wqkv_d = nc.declare_dram_parameter("w_qkv", [128, 54 * 128], BF16, isOutput=False)
    bqk_d = nc.declare_dram_parameter("b_qk", [768], F32, isOutput=False)
    bv_d = nc.declare_dram_parameter("b_v", [HG_DIM], BF16, isOutput=False)
    wproj_d = nc.declare_dram_parameter("w_proj", [HG_DIM, N_EMBD], BF16, isOutput=False)
    ones_d = nc.declare_dram_parameter("ones", [1, 128], BF16, isOutput=False)
    # y partials in bf16: halves the store traffic (the host sums the two
    # head-group partials in fp32; bf16 partial rounding adds ~0.1% error)
    y_d = nc.declare_dram_parameter("y", [S, N_EMBD], BF16, isOutput=True)

    with tile.TileContext(nc) as tc:
        from contextlib import ExitStack

        with ExitStack() as outer:
            consts = outer.enter_context(tc.tile_pool(name="consts", bufs=1))
            ones_row = consts.tile([1, 128], BF16)
            bias_v = consts.tile([1, HG_DIM], BF16)
            if not skip_vbias:
                nc.gpsimd.dma_start(out=ones_row[:], in_=ones_d[:])
                nc.gpsimd.dma_start(
                    out=bias_v[:], in_=bv_d[0:HG_DIM].rearrange("(o v) -> o v", o=1)
                )
            bias_qk = consts.tile([128, 6], F32)      # col m: b_qk[128m:128m+128]
            nc.gpsimd.dma_start(
                out=bias_qk[:], in_=bqk_d[0:768].rearrange("(m p) -> p m", p=128)
            )

            # ---- persistent activations/weights in SBUF (all bf16) ----
            big = outer.enter_context(tc.tile_pool(name="big", bufs=1))
            xT = big.tile([128, 6 * S], BF16)       # [emb-part, k-chunk*2048+seq]
            w_all = big.tile([128, 54 * 128], BF16)  # packed layout (see wqkv_d)

            def wcol(k, which):
                # column of 128-wide weight block `which` of k-chunk k in the
                # packed w_all layout: {m0,m3} k-major (cols 0:1536), then
                # pair0-v blocks (1536:2304), then the rest
                if which == "m0":
                    return k * 256
                if which == "m3":
                    return k * 256 + 128
                if which == "v0":
                    return 1536 + k * 128
                ri = {"m1": 0, "m2": 1, "m4": 2, "m5": 3, "v1": 4, "v2": 5}
                return 2304 + k * 768 + ri[which] * 128
            w_proj = big.tile([128, N_PAIRS * N_EMBD], BF16)
            qkT = big.tile([128, 6 * S], BF16)      # m=0..2 qT pairs, m=3..5 kT pairs
            # per k-tile: 6 heads x (64 v-cols + a ones col for the softmax
            # denominator) -> P@V and row-sums come from one M=65 matmul
            v_all = big.tile([128, ST * 390], BF16)  # [seq, t*390 + 65h + d]
            attnT = big.tile([128, N_PAIRS * S], BF16)

            # scratch operand for the PE warm-up matmuls below
            warm = consts.tile([128, 512], BF16)
            nc.gpsimd.memset(warm[:], 1.0)
            nc.gpsimd.memset(v_all[:], 1.0)
            # causal 128x128 triangle mask (1 where q-col >= k-row), built
            # once: the per-diag-tile masking is a DVE multiply by this tile
            # instead of a gpsimd affine_select -- affine_selects queued
            # BEHIND the previous pair's normalize partition_broadcast on the
            # strict-FIFO gpsimd at every pair boundary, stalling av(j0) ~5us.
            trimask = consts.tile([128, 128], BF16)
            nc.gpsimd.memset(trimask[:], 1.0)
            nc.gpsimd.affine_select(
                out=trimask[:], in_=trimask[:],
                compare_op=mybir.AluOpType.is_ge,
                fill=0.0, base=0, pattern=[[1, 128]], channel_multiplier=-1,
            )
            # CRITICAL lead-in inputs as SINGLE multi-dim strided DMAs (each
            # dma_start costs ~0.6us of ISSUE time on its trigger engine, so
            # issue count is what matters): the w columns the first attention
            # group needs ({0:128 m0-qT, 384:512 m3-kT, 768:896 pair0-v} per
            # k-chunk) in one DMA on sync, and the xT g0 quarter (cols 0:512
            # of every k-chunk) in one DMA on scalar, in parallel.
            # qT/kT weights first ({m0,m3}, in chunk halves so the first
            # matmuls start as soon as chunks 0-2 land), then the pair0-v
            # blocks (not read until av(j=0) at ~19us)
            nc.sync.dma_start(out=w_all[:, 0:768], in_=wqkv_d[:, 0:768])
            nc.sync.dma_start(out=w_all[:, 768:1536], in_=wqkv_d[:, 768:1536])
            nc.sync.dma_start(out=w_all[:, 1536:2304], in_=wqkv_d[:, 1536:2304])
            xT_view_s = xT[:].rearrange("p (k s) -> p k s", k=6)
            nc.scalar.dma_start(out=xT_view_s[:, 0:3, 0:512],
                                in_=xT_d[:, 0:1536])
            nc.scalar.dma_start(out=xT_view_s[:, 3:6, 0:512],
                                in_=xT_d[:, 1536:3072])


            # deferred inputs (one contiguous DMA each), deadline-queued on
            # sync behind the critical lead-in transfers.
            def emit_w_rest():
                nc.sync.dma_start(out=w_all[:, 2304:6912], in_=wqkv_d[:, 2304:6912])

            def emit_xT_quarter(g):
                nc.sync.dma_start(out=xT_view_s[:, :, g * 512:(g + 1) * 512],
                                  in_=xT_d[:, g * 3072:(g + 1) * 3072])

            def emit_wproj():
                nc.sync.dma_start(
                    out=w_proj[:].rearrange("p (c e) -> p c e", c=3),
                    in_=wproj_d[:].rearrange("(c p) e -> p c e", p=128),
                )

            # ---- pools ----
            stps = outer.enter_context(tc.tile_pool(name="stps", bufs=2, space="PSUM"))
            avps = outer.enter_context(tc.tile_pool(name="avps", bufs=2, space="PSUM"))
            auxps = outer.enter_context(tc.tile_pool(name="auxps", bufs=2, space="PSUM"))
            ptp = outer.enter_context(tc.tile_pool(name="ptp", bufs=6))
            avsb = outer.enter_context(tc.tile_pool(name="avsb", bufs=4))
            rcp = outer.enter_context(tc.tile_pool(name="rcp", bufs=6))
            bcp = outer.enter_context(tc.tile_pool(name="bcp", bufs=6))
            shtmp = outer.enter_context(tc.tile_pool(name="shtmp", bufs=3))
            ystage = outer.enter_context(tc.tile_pool(name="ystage", bufs=3))

            # PE clock warm-up: the HAM gate holds the PE at 1.2 GHz until
            # ~3.4us of sustained activity, and the PE is idle from the
            # preamble until the critical DMAs land (~13.9us).  Ten dummy
            # matmuls (run 9.5-13.8us, traced) warm the clock so the real
            # qk chains run at 2.4 GHz (216ns vs 426ns per matmul).
            wps = auxps.tile([128, 512], F32, tag="aux")
            for _ in range(10):
                nc.tensor.matmul(wps[:], warm[:, 0:128], warm[:],
                                 start=True, stop=True)

            v_view = v_all[:].rearrange("p (t h c) -> p t h c", t=ST, h=HG_HEADS)

            # ---- work-unit emitters (each emits a small PE-dense chunk) ----
            def emit_qk_group(m, g):
                # qkT[:, m*S + g*512 : +512] = (W[:, m-block].T @ xT)[:, g-block] + bias
                ps = auxps.tile([128, 512], F32, tag="aux")
                for k in range(6):
                    wc = wcol(k, f"m{m}")
                    nc.tensor.matmul(
                        ps[:],
                        w_all[:, wc:wc + 128],
                        xT[:, k * S + g * 512:k * S + (g + 1) * 512],
                        start=(k == 0), stop=(k == 5),
                    )
                nc.vector.tensor_scalar_add(
                    qkT[:, m * S + g * 512:m * S + (g + 1) * 512],
                    ps[:], bias_qk[:, m:m + 1],
                )

            def emit_v_tile(pair, t):
                # v rows t*128.. for this pair's two heads (N=128); split by
                # pair so each attention slot computes only its own v work
                ps = auxps.tile([128, 128], F32, tag="aux")
                for k in range(6):
                    wc = wcol(k, f"v{pair}")
                    nc.tensor.matmul(
                        ps[:],
                        xT[:, k * S + t * 128:k * S + (t + 1) * 128],
                        w_all[:, wc:wc + 128],
                        start=(k == 0), stop=(skip_vbias and k == 5),
                    )
                if not skip_vbias:
                    nc.tensor.matmul(   # += ones^T[1,128].T @ bias_v[1,128]
                        ps[:], ones_row[:],
                        bias_v[:, pair * 128:(pair + 1) * 128],
                        start=False, stop=True,
                    )
                nc.vector.tensor_copy(
                    v_view[:, t, 2 * pair:2 * pair + 2, 0:64],
                    ps[:].rearrange("p (h d) -> p h d", h=2),
                )

            ys_pending = {}
            drain_ps = {}

            def emit_proj_partial(t, use_avps):
                # drain phase: pairs 0/1 of a proj tile accumulate while the
                # final normalize chain resolves (pair 2 would block the
                # strict-FIFO PE queue).  Tile t8 borrows the now-free AV
                # PSUM banks so two tiles can be in flight alongside the
                # aux pool.
                pool = avps if use_avps else auxps
                tag = "av" if use_avps else "aux"
                psA = pool.tile([128, 512], F32, tag=tag)
                psB = pool.tile([128, 256], F32, tag=tag)
                for p in range(2):
                    lhsT = attnT[:, p * S + t * 128:p * S + (t + 1) * 128]
                    nc.tensor.matmul(psA[:], lhsT, w_proj[:, p * N_EMBD:p * N_EMBD + 512],
                                     start=(p == 0), stop=False)
                    nc.tensor.matmul(psB[:], lhsT,
                                     w_proj[:, p * N_EMBD + 512:(p + 1) * N_EMBD],
                                     start=(p == 0), stop=False)
                drain_ps[t] = (psA, psB)

            def emit_proj_finish(t):
                psA, psB = drain_ps.pop(t)
                lhsT = attnT[:, 2 * S + t * 128:2 * S + (t + 1) * 128]
                nc.tensor.matmul(psA[:], lhsT, w_proj[:, 2 * N_EMBD:2 * N_EMBD + 512],
                                 start=False, stop=True)
                nc.tensor.matmul(psB[:], lhsT,
                                 w_proj[:, 2 * N_EMBD + 512:3 * N_EMBD],
                                 start=False, stop=True)
                ys = ystage.tile([128, 2 * N_EMBD], BF16, tag="ys")
                AFc = mybir.ActivationFunctionType.Copy
                nc.scalar.activation(ys[:, 0:512], psA[:], AFc)
                nc.scalar.activation(ys[:, 512:768], psB[:], AFc)
                nc.sync.dma_start(out=y_d[t * 128:(t + 1) * 128, :],
                                  in_=ys[:, 0:768])

            def emit_proj_tile(t, drain=False):
                # stage into the left/right half of a 2-tile ystage buffer;
                # the odd tile of each pair issues one batched y DMA.  In the
                # post-exp drain the PSUM->stage copies run on the (now idle)
                # ACT engine so they never queue behind DVE normalize work.
                psA = auxps.tile([128, 512], F32, tag="aux")
                psB = auxps.tile([128, 256], F32, tag="aux")
                for p in range(N_PAIRS):
                    lhsT = attnT[:, p * S + t * 128:p * S + (t + 1) * 128]
                    nc.tensor.matmul(psA[:], lhsT, w_proj[:, p * N_EMBD:p * N_EMBD + 512],
                                     start=(p == 0), stop=(p == N_PAIRS - 1))
                    nc.tensor.matmul(psB[:], lhsT,
                                     w_proj[:, p * N_EMBD + 512:(p + 1) * N_EMBD],
                                     start=(p == 0), stop=(p == N_PAIRS - 1))
                if drain:
                    # drain phase: per-tile stores (a 2-tile batch would hold
                    # the last store until both tiles finish) and ACT copies
                    # (the DVE is busy with the final normalize)
                    ys = ystage.tile([128, 2 * N_EMBD], BF16, tag="ys")
                    AFc = mybir.ActivationFunctionType.Copy
                    nc.scalar.activation(ys[:, 0:512], psA[:], AFc)
                    nc.scalar.activation(ys[:, 512:768], psB[:], AFc)
                    nc.sync.dma_start(out=y_d[t * 128:(t + 1) * 128, :],
                                      in_=ys[:, 0:768])
                    return
                if t % 2 == 0:
                    ys = ystage.tile([128, 2 * N_EMBD], BF16, tag="ys")
                    ys_pending[t] = ys
                else:
                    ys = ys_pending.pop(t - 1)
                half = (t % 2) * N_EMBD
                nc.vector.tensor_copy(ys[:, half:half + 512], psA[:])
                nc.vector.tensor_copy(ys[:, half + 512:half + 768], psB[:])
                if t % 2 == 1:
                    b = t // 2
                    nc.sync.dma_start(
                        out=y_d[b * 256:(b + 1) * 256, :]
                            .rearrange("(i p) e -> p i e", p=128),
                        in_=ys[:].rearrange("p (i e) -> p i e", i=2),
                    )

            # ---- deadline-driven background work queue ----
            # Attention groups execute in a fixed order; (pair, g, j) maps to
            # a global step.  Each qkv/proj work unit carries the step by
            # which it MUST be emitted (Tile deps are emission-order-based:
            # a read emitted before its producer gets no dependency).  Units
            # are pulled with LOOKAHEAD steps of slack so the PE always has
            # background matmuls to chew on while ACT runs exp.
            # pair-2 groups run [1,0,3,2]: each group's normalize + proj
            # tiles emit early in the FOLLOWING group (tight deadlines), so
            # after the last exp only group g2's normalize + proj t8-11
            # remain.
            group_order = {0: [0, 1, 2, 3], 1: [0, 1, 2, 3], 2: [1, 0, 3, 2]}
            step_base = {}
            _acc = 0
            for _p in range(N_PAIRS):
                for _g in group_order[_p]:
                    step_base[(_p, _g)] = _acc
                    _acc += 4 * _g + 4
            TOTAL_STEPS = _acc
            LOOKAHEAD = 32

            work_q = []   # sorted list of (deadline_step, seq, fn)
            _seq = [0]

            def push(deadline, fn):
                import bisect
                _seq[0] += 1
                bisect.insort(work_q, (deadline, _seq[0], fn))

            def pull_work(cur_step):
                # overdue units MUST emit now (correctness: emission order
                # defines Tile dependencies); otherwise spread at one unit
                # per step so the background work stays evenly interleaved.
                while work_q and work_q[0][0] <= cur_step:
                    work_q.pop(0)[2]()
                for _ in range(2):
                    if work_q and work_q[0][0] <= cur_step + LOOKAHEAD:
                        work_q.pop(0)[2]()

            # ---- attention group with interleaved background units ----
            sts_all = {}
            pts_all = {}

            def scores_pg(pair, g, j):
                q0 = pair * S
                k0 = (3 + pair) * S
                diag_r = j - 4 * g
                c0 = 128 * diag_r if diag_r >= 0 else 0
                st = stps.tile([128, 1024], F32, tag="st")
                nc.tensor.matmul(
                    st[:, c0:512],
                    qkT[0:64, k0 + j * 128:k0 + (j + 1) * 128],
                    qkT[0:64, q0 + g * 512 + c0:q0 + (g + 1) * 512],
                    start=True, stop=True, tile_position=(0, 0),
                )
                nc.tensor.matmul(
                    st[:, 512 + c0:1024],
                    qkT[64:128, k0 + j * 128:k0 + (j + 1) * 128],
                    qkT[64:128, q0 + g * 512 + c0:q0 + (g + 1) * 512],
                    start=True, stop=True, tile_position=(64, 0),
                )
                sts_all[(pair, g, j)] = (st, c0)

            def expmask_pg(pair, g, j):
                st, c0 = sts_all.pop((pair, g, j))
                pt = ptp.tile([128, 1024], BF16, tag="pt")
                if c0 >= 256:
                    # deep-diagonal tile: head1's [512:512+c0] range is never
                    # read by its AV matmul, so exp the two live ranges
                    # separately (saves c0*128 ACT elements, > the ~170ns
                    # extra instruction cost once c0 >= 256)
                    nc.scalar.activation(pt[:, c0:512], st[:, c0:512],
                                         AF.Exp, bias=0.0, scale=0.125)
                    nc.scalar.activation(pt[:, 512 + c0:1024], st[:, 512 + c0:1024],
                                         AF.Exp, bias=0.0, scale=0.125)
                else:
                    nc.scalar.activation(pt[:, c0:1024], st[:, c0:1024],
                                         AF.Exp, bias=0.0, scale=0.125)
                diag_r = j - 4 * g
                if diag_r >= 0:
                    with nc.allow_low_precision(reason="bf16 causal mask"):
                        for h in range(2):
                            nc.vector.tensor_mul(
                                pt[:, h * 512 + c0:h * 512 + c0 + 128],
                                pt[:, h * 512 + c0:h * 512 + c0 + 128],
                                trimask[:],
                            )
                pts_all[(pair, g, j)] = (pt, c0)

            def emit_attn_group(pair, g, nxt_ctx=None):
                njt = 4 * g + 4
                av0 = avps.tile([65, 512], F32, tag="av")
                av1 = avps.tile([65, 512], F32, tag="av")

                def av(j):
                    pt, c0 = pts_all.pop((pair, g, j))
                    first, last = (j == 0), (j == njt - 1)
                    for h, avt in ((0, av0), (1, av1)):
                        nc.tensor.matmul(
                            avt[0:65, c0:512],
                            v_all[:, j * 390 + (2 * pair + h) * 65:
                                  j * 390 + (2 * pair + h) * 65 + 65],
                            pt[:, h * 512 + c0:(h + 1) * 512],
                            start=first, stop=last,
                        )

                if (pair, g, 0) not in pts_all:
                    scores_pg(pair, g, 0)
                    expmask_pg(pair, g, 0)
                base = step_base[(pair, g)]
                for j in range(njt):
                    if j + 1 < njt:
                        scores_pg(pair, g, j + 1)
                        expmask_pg(pair, g, j + 1)
                    pull_work(base + j)
                    av(j)

                # evacuate the AV accumulators to SBUF (one copy per head into
                # a shared [65,1024] staging tile -- frees the PSUM banks for
                # the next group's AV almost immediately); the
                # recip/redistribute/multiply chain is DEFERRED into the next
                # group's instruction stream so it never stalls the PE at the
                # group boundary.
                avs = avsb.tile([65, 1024], F32, tag="avsb")

                def evacuate():
                    if g == group_order[pair][-1]:
                        # pair boundary: evacuate on ACT (it idles there
                        # while the DVE works the normalize chain)
                        AFc = mybir.ActivationFunctionType.Copy
                        nc.scalar.activation(avs[:, 0:512], av0[:], AFc)
                        nc.scalar.activation(avs[:, 512:1024], av1[:], AFc)
                    else:
                        nc.vector.tensor_copy(avs[:, 0:512], av0[:])
                        nc.vector.tensor_copy(avs[:, 512:1024], av1[:])

                final = (pair == 2 and g == group_order[2][-1])
                if final:
                    evacuate()
                else:
                    # deferred to the NEXT group's step 0: the copies then
                    # queue BEHIND its first exp in the engine FIFO (emitted
                    # inline they delayed that exp ~1.2us at pair
                    # boundaries) but still ahead of its av(0), which needs
                    # these PSUM banks back.
                    push(base + njt, evacuate)

                # with the consolidated input DMAs the sync queue is near
                # idle mid-kernel, so all normalize DMAs ride it (HWDGE; the
                # gpsimd SWDGE path costs ~1us + library reloads and stalled
                # the chain behind affine_selects in practice).
                dma_eng = nc.sync

                def normalize():
                    cols = slice(pair * S + g * 512, pair * S + (g + 1) * 512)
                    # DVE reciprocal runs ~9 cyc/elem PER LANE: on [1,1024]
                    # it would cost ~6us.  Reshape both heads' denominators
                    # to [128,8] via ONE SBUF DMA (flat row-major pairing:
                    # partition p <- cols 8p..8p+7, so p<64 is head0) so the
                    # recip uses 128 lanes (~0.2us), then shape back to
                    # [1,1024] for the gpsimd partition broadcasts.
                    dn8 = rcp.tile([128, 8], F32, tag="dn8")
                    dma_eng.dma_start(out=dn8[:], in_=avs[64:65, :])
                    if final:
                        # head1's UNNORMALIZED rows shifted to partitions
                        # 64-127 in parallel with the reciprocal chain; the
                        # in-place multiply below reads the PE broadcast at
                        # partitions 64-127 directly, cutting the
                        # mult->shift->sem tail (~2.6us measured) off the
                        # final critical path.
                        sh = shtmp.tile([128, 512], F32, tag="sh")
                        dma_eng.dma_start(out=sh[64:128, :],
                                          in_=avs[0:64, 512:1024])
                    with nc.allow_low_precision(reason="softmax normalize bf16"):
                        if final:
                            # drain phase: the PE is idle and the scores PSUM
                            # pool is free, so broadcast the reciprocals with
                            # a rank-1 matmul (trimask row 0 is all-ones)
                            # instead of two serial ~1us partition_broadcasts
                            # -- and head1's multiply reads the PSUM
                            # broadcast directly (every row is identical), so
                            # no gpsimd at all on the final critical chain.
                            rc8b = rcp.tile([128, 8], BF16, tag="rc8b")
                            nc.vector.reciprocal(rc8b[:], dn8[:])
                            rc2b = rcp.tile([1, 1024], BF16, tag="rc2b")
                            dma_eng.dma_start(out=rc2b[:], in_=rc8b[:])
                            bc_ps = stps.tile([128, 1024], F32, tag="st")
                            for h in range(2):
                                nc.tensor.matmul(
                                    bc_ps[:, h * 512:(h + 1) * 512],
                                    trimask[0:1, 0:128],
                                    rc2b[:, h * 512:(h + 1) * 512],
                                    start=True, stop=True,
                                )
                            # multiply in column halves: the finish of
                            # proj tiles t8/t9 only reads the first 256
                            # normalized columns, and range-based deps let
                            # it start ~0.7us before the full-width multiply
                            # would allow
                            q0 = pair * S + g * 512
                            for lo, hi in ((0, 256), (256, 512)):
                                nc.vector.tensor_mul(
                                    attnT[0:64, q0 + lo:q0 + hi],
                                    avs[0:64, lo:hi], bc_ps[0:64, lo:hi])
                                nc.vector.tensor_mul(
                                    attnT[64:128, q0 + lo:q0 + hi],
                                    sh[64:128, lo:hi],
                                    bc_ps[64:128, 512 + lo:512 + hi])
                            return
                        rc8 = rcp.tile([128, 8], F32, tag="rc8")
                        nc.vector.reciprocal(rc8[:], dn8[:])
                        rc2 = rcp.tile([1, 1024], F32, tag="rc2")
                        dma_eng.dma_start(out=rc2[:], in_=rc8[:])
                        for h in range(2):
                            bc = bcp.tile([64, 512], F32)
                            nc.gpsimd.partition_broadcast(
                                bc[:], rc2[:, h * 512:(h + 1) * 512], channels=64)
                            if h == 0:
                                nc.vector.tensor_mul(attnT[0:64, cols],
                                                     avs[0:64, 0:512], bc[:])
                            else:
                                # DVE lanes are partition-locked: odd head's
                                # rows 64-127 via an SBUF bounce + DMA shift
                                tmp = shtmp.tile([64, 512], BF16)
                                nc.vector.tensor_mul(tmp[:], avs[0:64, 512:1024],
                                                     bc[:])
                                nc.sync.dma_start(out=attnT[64:128, cols],
                                                  in_=tmp[:])

                nxt = base + njt
                if pair == 2:
                    # tight deadlines: normalize pops at the next group's
                    # step 0 (eligible from nxt+1-LOOKAHEAD, head of queue by
                    # (deadline, seq)), proj tiles follow one per step.  For
                    # the final group nxt == TOTAL_STEPS and these drain
                    # immediately after the j-loop, in push order.
                    push(nxt + 1, normalize)
                    if final:
                        t0 = 4 * g
                        push(nxt + 2, lambda: emit_proj_partial(t0, True))
                        push(nxt + 3, lambda: emit_proj_partial(t0 + 1, False))
                        push(nxt + 4, lambda: emit_proj_finish(t0))
                        push(nxt + 5, lambda: emit_proj_finish(t0 + 1))
                        push(nxt + 6, lambda: emit_proj_tile(t0 + 2, drain=True))
                        push(nxt + 7, lambda: emit_proj_tile(t0 + 3, drain=True))
                    else:
                        for i, t in enumerate(range(4 * g, 4 * g + 4)):
                            push(nxt + 2 + i, lambda t=t: emit_proj_tile(t))
                else:
                    # pairs 0/1: keep the relaxed deadline so the broadcast
                    # queues behind the next group's first affine_selects.
                    push(nxt + LOOKAHEAD, normalize)

            # ================= schedule =================
            # upfront: just enough qkv for attn(0, g0); v t0-3 go through
            # the deadline queue (first read at av(j=t) of group (0,0))
            emit_qk_group(3, 0)          # kT pair 0, seq 0-511
            emit_qk_group(0, 0)          # qT pair 0, seq 0-511

            # deadlines: qT(p, g) is read only by group (p, g); kT(p, g') is
            # read by EVERY group (p, g >= g'), so its deadline is the
            # earliest-executing such group - for pair 2 (non-monotone group
            # order) that is the first group of the pair for ALL kT chunks.
            # qT/kT for pairs 1/2 run THREE steps early: emitted
            # just-in-time (base-1) the next group's first scores wait ~2us
            # for the qk chain + bias add, stalling the exp stream at every
            # group boundary.  Pair 0's stay just-in-time: its xT quarters
            # are still IN FLIGHT, and emitting compute against an un-landed
            # DMA parks a PSUM buffer + the strict-FIFO PE queue on it
            # (measured +40us!).
            for p in range(N_PAIRS):
                for g in range(NG):
                    if (p, g) == (0, 0):
                        continue
                    slack = 3 if p > 0 else 1
                    kt_dl = min(step_base[(p, gg)] for gg in range(g, NG)) - slack
                    push(kt_dl, lambda m=3 + p, g=g: emit_qk_group(m, g))
                    push(step_base[(p, g)] - slack,
                         lambda m=p, g=g: emit_qk_group(m, g))
            # v(pair, t) is first read at av(j=t) of the earliest-executing
            # group g of that pair with 4g+3 >= t
            for p in range(N_PAIRS):
                for t in range(16):
                    dl = min(step_base[(p, g)]
                             for g in group_order[p] if 4 * g + 3 >= t) + t
                    push(dl, lambda p=p, t=t: emit_v_tile(p, t))
            # deferred w complement: needed first by pair-1 qT/kT/v work
            # (earliest deadline around step_base[(1,0)]-3)
            push(step_base[(0, 2)], emit_w_rest)
            # xT quarter g is first read by qk(0, g) units (deadline base-1)
            for g in range(1, NG):
                push(step_base[(0, g)] - 2, lambda g=g: emit_xT_quarter(g))
            # w_proj is first read by proj units in pair 2
            push(step_base[(1, 0)], emit_wproj)

            seq = [(p, g) for p in range(N_PAIRS) for g in group_order[p]]
            for i, (pair, g) in enumerate(seq):
                nxt_ctx = seq[i + 1] if i + 1 < len(seq) else None
                emit_attn_group(pair, g, nxt_ctx)

            # drain in deadline order: the final group's normalize precedes
            # its proj tiles (same-ordered deadlines)
            while work_q:
                work_q.pop(0)[2]()

    nc.compile()
    return nc


def _numpy_fallback(x, mask, W_attn, b_attn, W_proj, b_proj):
    qkv = x @ W_attn + b_attn
    q, k, v = np.split(qkv, 3, axis=-1)

    def heads(t):
        return t.reshape(B, S, N_HEAD, HEAD_DIM).transpose(0, 2, 1, 3)

    q, k, v = heads(q), heads(k), heads(v)
    attn = np.einsum("bhqd,bhkd->bhqk", q, k) / np.sqrt(np.float32(HEAD_DIM))
    attn = attn + mask * (-1e9)
    attn = attn - attn.max(axis=-1, keepdims=True)
    attn = np.exp(attn)
    attn = attn / attn.sum(axis=-1, keepdims=True)
    out = np.einsum("bhqk,bhkd->bhqd", attn, v)
    out = out.transpose(0, 2, 1, 3).reshape(B, S, N_EMBD)
    return (out @ W_proj + b_proj).astype(np.float32)


def _pack_w(Wc):
    """[768, 1152] per-core qkv weight -> [128, 6912] packed layout: cols
    0:2304 = k-major {m0, m3, v0} blocks (the first attention group's
    critical columns), cols 2304:6912 = k-major {m1, m2, m4, m5, v1, v2}."""
    critA = np.concatenate([Wc[:, 0:128], Wc[:, 384:512]], axis=1)  # [768, 256]
    critAP = critA.reshape(6, 128, 256).transpose(1, 0, 2).reshape(128, 1536)
    vblkP = Wc[:, 768:896].reshape(6, 128, 128).transpose(1, 0, 2).reshape(128, 768)
    rest = np.concatenate(
        [Wc[:, 128:384], Wc[:, 512:768], Wc[:, 896:1152]], axis=1)   # [768, 768]
    restP = rest.reshape(6, 128, 768).transpose(1, 0, 2).reshape(128, 4608)
    return np.concatenate([critAP, vblkP, restP], axis=1)


def make_in_maps(x, W_attn, b_attn, W_proj):
    bf16 = ml_dtypes.bfloat16
    in_maps = []
    for c in range(N_CORES):
        b, hg = divmod(c, 2)
        o = HG_DIM * hg
        Wc = np.concatenate(
            [W_attn[:, o:o + HG_DIM],
             W_attn[:, 768 + o:768 + o + HG_DIM],
             W_attn[:, 1536 + o:1536 + o + HG_DIM]], axis=1)
        xTc = x[b].T.astype(bf16)   # [768, 2048]
        xT_packed = (xTc.reshape(6, 128, 4, 512).transpose(1, 2, 0, 3)
                     .reshape(128, 4 * 3072))
        in_maps.append({
            "xT": np.ascontiguousarray(xT_packed),
            "w_qkv": np.ascontiguousarray(_pack_w(Wc).astype(bf16)),
            "b_qk": np.ascontiguousarray(np.concatenate(
                [b_attn[o:o + HG_DIM], b_attn[768 + o:768 + o + HG_DIM]])),
            "b_v": np.ascontiguousarray(b_attn[1536 + o:1536 + o + HG_DIM]).astype(bf16),
            "w_proj": np.ascontiguousarray(W_proj[o:o + HG_DIM, :].astype(bf16)),
            "ones": np.ones((1, 128), dtype=bf16),
        })
    return in_maps


def kernel(x, mask, W_attn, b_attn, W_proj, b_proj):
    global LAST_RESULTS
    x = np.asarray(x, dtype=np.float32)
    mask = np.asarray(mask, dtype=np.float32)
    W_attn = np.asarray(W_attn, dtype=np.float32)
    b_attn = np.asarray(b_attn, dtype=np.float32)
    W_proj = np.asarray(W_proj, dtype=np.float32)
    b_proj = np.asarray(b_proj, dtype=np.float32)

    # the kernel exploits causal structure; verify the mask actually is causal
    causal = 1.0 - np.tril(np.ones((S, S), dtype=np.float32))
    if mask.shape != (1, 1, S, S) or not np.array_equal(mask[0, 0], causal):
        return _numpy_fallback(x, mask, W_attn, b_attn, W_proj, b_proj)

    from concourse.bass_utils import run_bass_kernel_spmd

    skip_vbias = not b_attn[1536:2304].any()   # v-bias exactly zero
    if skip_vbias not in _PROGRAMS:
        _PROGRAMS[skip_vbias] = _build_program(skip_vbias=skip_vbias)

    in_maps = make_in_maps(x, W_attn, b_attn, W_proj)

    trace = bool(int(os.environ.get("ATTN_KERNEL_TRACE", "0")))
    res = run_bass_kernel_spmd(_PROGRAMS[skip_vbias], in_maps,
                               list(range(N_CORES)), trace=trace)
    LAST_RESULTS = res

    y = np.zeros((B, S, N_EMBD), dtype=np.float32)
    for c in range(N_CORES):
        y[c // 2] += res.results[c]["y"].astype(np.float32)
    y += b_proj
    return y
